# revision 1
# baseline (speedup 1.0000x reference)
"""Trainium2 Bass kernel for GQA decode attention (B=32,T=4,D=2048,H=16,G=4,K=128,S=4096).

Sharding: 8 NeuronCores = 2 batch-groups x 4 kv-head groups.
Core c: batches [16*(c//4), 16*(c//4)+16), kv head g = c % 4 (owns 4 q heads).
o_proj partial sums are AllReduce'd across each 4-core head group on device.

Device pipeline per core:
  - QKV projection (fp16 matmuls, fp32 PSUM accumulate)
  - RMS-norm + RoPE in fp32 on DVE/ACT (host-precomputed coefficient tables
    with q_scale/k_scale folded in)
  - attention in transposed orientation: PE-transpose K tiles, logits^T =
    kT @ qT (fp16 in, fp32 out), softmax WITHOUT max-subtraction -- safe
    because rms-normed q,k bound |logits| <= sqrt(K); exp computes
    exp(x/sqrt(K) - 1) on ACT (the -1 cancels in normalization and keeps
    fp16 exp values < 65504)
  - A.V accumulates attn^T directly (v natural layout as the stationary)
  - fp16 o_proj into fp32 PSUM, AllReduce, DMA out

Only cache rows [0, cur_ind) are read; rows [cur_ind, cur_ind+T) are the
freshly projected k/v handled on-chip, rows beyond are masked by the
reference -- so the cache update never materializes.
"""

import sys

sys.path.insert(0, "/opt/trn_rl_repo")

import numpy as np

import concourse.bacc as bacc
import concourse.mybir as mybir
import concourse.tile as tile
from concourse.bass_utils import run_bass_kernel_spmd

F32 = mybir.dt.float32
F32R = mybir.dt.float32r
F16 = mybir.dt.float16

B, T, D = 32, 4, 2048
H, G, K = 16, 4, 128
S = 4096
R = H // G          # 4 q heads per kv head
EPS = 1e-6
ROPE_BASE = 10000.0
NCORES = 8
BG = 2              # batch groups
BL = B // BG        # 16 batches per core
TOK = BL * T        # 64 tokens per core
QCOLS = R * K       # 512 local q columns
NDC = D // 128      # 16 contraction chunks for qkv proj
SCALE = 1.0 / np.sqrt(np.float32(K))
EXP_BIAS = -1.0     # exp(x*SCALE + EXP_BIAS); cancels in softmax, avoids fp16 overflow

_COMPILED = {}


def _pack_rows(w):
    """(C*128, N) -> (128, C*N) with [p, c*N+n] = w[c*128+p, n]."""
    c = w.shape[0] // 128
    n = w.shape[1]
    return np.ascontiguousarray(
        w.reshape(c, 128, n).transpose(1, 0, 2).reshape(128, c * n)
    )


def _build_nc(cur, n_tiles, use_collective):
    nc = bacc.Bacc("TRN2", target_bir_lowering=False, debug=False, num_devices=NCORES)

    ext = {}

    def inp(name, shape, dt=F32):
        ext[name] = nc.dram_tensor(name, list(shape), dt, kind="ExternalInput")
        return ext[name]

    inp("hT", (128, NDC * TOK), F16)       # hiddenT packed per d-chunk
    inp("wq", (128, NDC * QCOLS), F16)
    inp("wk", (128, NDC * K), F16)
    inp("wv", (128, NDC * K), F16)
    inp("wo", (128, R * D), F16)
    inp("kc", (BL, 128, n_tiles * K), F16) # cache, host-packed (p = s%128), fp16
    inp("vc", (BL, 128, n_tiles * K), F16)
    inp("aq", (TOK, QCOLS))
    inp("bq", (TOK, QCOLS))
    inp("ak", (TOK, K))
    inp("bk", (TOK, K))
    inp("ident16", (128, 128), F16)
    inp("ones16", (128, 1), F16)
    inp("maskf", (T, BL * 4 * T), F16)     # multiplicative causal mask for fresh tokens
    out_ext = nc.dram_tensor("out", [TOK, D], F32, kind="ExternalOutput")

    NQROW = 4 * T                          # 16 query rows per batch (r*4+t)
    LCOLS = n_tiles * NQROW                # logitsT bank cols per batch

    with tile.TileContext(nc) as tc:
        from contextlib import ExitStack

        with ExitStack() as ctx:
            cpool = ctx.enter_context(tc.tile_pool(name="const", bufs=1))

            def load(name, dt=None, eng=None, split=1):
                h = ext[name]
                t_ = cpool.tile(list(h.shape), dt or h.dtype, tag=name)
                ncols = h.shape[-1]
                step = ncols // split
                for s0 in range(0, ncols, step):
                    (eng or nc.sync).dma_start(t_[:, s0:s0 + step], h.ap()[:, s0:s0 + step])
                return t_

            hT = load("hT", split=4)
            wq = load("wq", split=4)
            wk = load("wk", split=4)
            wv = load("wv", split=4)
            wo = load("wo", split=2)
            aq = load("aq")
            bq = load("bq")
            ak = load("ak")
            bk = load("bk")
            ident16 = load("ident16")
            ones16 = load("ones16")
            maskf = load("maskf")

            # ---------------- Phase 1: QKV projection + norm + rope ----------
            c_eps = cpool.tile([128, 1], F32, tag="c_eps")
            c_neg1 = cpool.tile([128, 1], F32, tag="c_neg1")
            nc.vector.memset(c_eps[:], float(EPS))
            nc.vector.memset(c_neg1[:], float(EXP_BIAS))
            ones_row = cpool.tile([1, 128], F32, tag="ones_row")
            nc.vector.memset(ones_row[:], 1.0)

            qn = cpool.tile([TOK, QCOLS], F32, tag="qn")       # normed+roped q
            kn = cpool.tile([TOK, K], F32, tag="kn")
            WQKV = QCOLS + 2 * K                               # 768 combined cols
            qkv16 = cpool.tile([TOK, WQKV], F16, tag="qkv16")
            q16 = qkv16[:, 0:QCOLS]
            k16n = qkv16[:, QCOLS:QCOLS + K]
            v16n = qkv16[:, QCOLS + K:WQKV]
            ssq = cpool.tile([TOK, 8], F32, tag="ssq")
            rstd = cpool.tile([TOK, 8], F32, tag="rstd")
            scr = cpool.tile([TOK, QCOLS], F32, tag="scr")
            scr2 = cpool.tile([TOK, QCOLS], F32, tag="scr2")

            with tc.tile_pool(name="ps1", bufs=1, space="PSUM") as ps1:
                pq = ps1.tile([TOK, QCOLS], F32, tag="pq")
                pk = ps1.tile([TOK, K], F32, tag="pk")
                pv = ps1.tile([TOK, K], F32, tag="pv")
                for c in range(NDC):
                    lhs = hT[:, c * TOK:(c + 1) * TOK]
                    st, sp = (c == 0), (c == NDC - 1)
                    nc.tensor.matmul(pq[:], lhs, wq[:, c * QCOLS:(c + 1) * QCOLS], start=st, stop=sp)
                    nc.tensor.matmul(pk[:], lhs, wk[:, c * K:(c + 1) * K], start=st, stop=sp)
                    nc.tensor.matmul(pv[:], lhs, wv[:, c * K:(c + 1) * K], start=st, stop=sp)

                # sum of squares per (token, head)
                for h in range(R):
                    nc.scalar.activation(scr[:, h * K:(h + 1) * K], pq[:, h * K:(h + 1) * K],
                                         mybir.ActivationFunctionType.Square,
                                         accum_out=ssq[:, h:h + 1])
                nc.scalar.activation(scr2[:, 0:K], pk[:],
                                     mybir.ActivationFunctionType.Square,
                                     accum_out=ssq[:, R:R + 1])
                # std = sqrt(ssq/K + eps); rstd = 1/std
                nc.scalar.activation(rstd[:, 0:5], ssq[:, 0:5],
                                     mybir.ActivationFunctionType.Sqrt,
                                     bias=c_eps[0:TOK, 0:1], scale=float(1.0 / K))
                nc.vector.reciprocal(ssq[:, 0:5], rstd[:, 0:5])
                # q_hat = q * rstd (per token/head), same for k
                for h in range(R):
                    nc.vector.tensor_scalar(qn[:, h * K:(h + 1) * K], pq[:, h * K:(h + 1) * K],
                                            ssq[:, h:h + 1], None, mybir.AluOpType.mult)
                nc.vector.tensor_scalar(kn[:], pk[:], ssq[:, R:R + 1], None, mybir.AluOpType.mult)
                # v -> fp16
                nc.scalar.copy(v16n, pv[:])

            # rope: out = qh*A + swap_halves(qh)*B   (scale folded into A/B)
            def rope(dst16, x, a, b, s1, s2, nh):
                xr = x[:].rearrange("p (h u x) -> p h u x", h=nh, u=2)
                # dst16 is an AP slice of the combined qkv16 tile
                br = b[:].rearrange("p (h u x) -> p h u x", h=nh, u=2)
                s2r = s2[:, 0:nh * K].rearrange("p (h u x) -> p h u x", h=nh, u=2)
                nc.vector.tensor_tensor(s1[:, 0:nh * K], x[:, 0:nh * K], a[:, 0:nh * K], mybir.AluOpType.mult)
                # swapped-half products
                nc.vector.tensor_tensor(s2r[:, :, 0, :], xr[:, :, 1, :], br[:, :, 0, :], mybir.AluOpType.mult)
                nc.vector.tensor_tensor(s2r[:, :, 1, :], xr[:, :, 0, :], br[:, :, 1, :], mybir.AluOpType.mult)
                nc.vector.tensor_tensor(s1[:, 0:nh * K], s1[:, 0:nh * K], s2[:, 0:nh * K], mybir.AluOpType.add)
                nc.vector.tensor_copy(dst16, s1[:, 0:nh * K])

            rope(q16, qn, aq, bq, scr, scr2, R)
            rope(k16n, kn, ak, bk, scr, scr2, 1)

            # flatten only v's (b t) partition layout -> t-partitions (base 0)
            vfl = cpool.tile([T, BL * K], F16, tag="vfl")
            for t in range(T):
                nc.gpsimd.dma_start(vfl[t:t + 1, :], v16n[t::T, :])

            def vfl_s(b):
                return vfl[0:T, b * K:(b + 1) * K]

            # transpose q -> qT (128k x 16 qrow per b), k_new -> kTnew (128k x 4 per b)
            # qT via 4 whole-column transposes (64 tokens at once, base-0 aligned),
            # then one copy permuting (r b t) -> (b r t); kTnew via one transpose.
            qT = cpool.tile([128, BL * NQROW], F16, tag="qT")
            kTnew = cpool.tile([128, BL * T], F16, tag="kTnew")
            with tc.tile_pool(name="ps2", bufs=1, space="PSUM") as ps2:
                qTp = ps2.tile([128, BL * NQROW], F16, tag="qTp")
                kTnp = ps2.tile([128, BL * T], F16, tag="kTnp")
                for r in range(R):
                    nc.tensor.matmul(qTp[:, r * TOK:(r + 1) * TOK],
                                     q16[:, r * K:(r + 1) * K],
                                     ident16[0:TOK, 0:TOK], is_transpose=True,
                                     skip_group_check=True)
                nc.tensor.matmul(kTnp[:], k16n,
                                 ident16[0:TOK, 0:TOK], is_transpose=True,
                                 skip_group_check=True)
                nc.scalar.copy(
                    qT[:].rearrange("p (b r t) -> p b r t", b=BL, r=R),
                    qTp[:].rearrange("p (r b t) -> p b r t", r=R, b=BL))
                nc.scalar.copy(kTnew[:], kTnp[:])

            # ---------------- Fresh-token logits / exp / denom ---------------
            expfr = cpool.tile([T, BL * NQROW], F16, tag="expfr")
            freshden = cpool.tile([1, BL * NQROW], F32, tag="freshden")
            with tc.tile_pool(name="ps3", bufs=1, space="PSUM") as ps3:
                frp = ps3.tile([T, BL * NQROW], F32, tag="frp")
                fdp = ps3.tile([1, BL * NQROW], F32, tag="fdp")
                for b in range(BL):
                    nc.tensor.matmul(frp[0:T, b * NQROW:(b + 1) * NQROW],
                                     kTnew[:, b * T:(b + 1) * T],
                                     qT[:, b * NQROW:(b + 1) * NQROW],
                                     skip_group_check=True)
                nc.scalar.activation(expfr[:], frp[:], mybir.ActivationFunctionType.Exp,
                                     bias=c_neg1[0:T, 0:1], scale=float(SCALE))
                nc.vector.tensor_tensor(expfr[:], expfr[:], maskf[:], mybir.AluOpType.mult)
                nc.tensor.matmul(fdp[:], ones16[0:T, 0:1], expfr[:])
                nc.scalar.copy(freshden[:], fdp[:])

            # ---------------- Main attention loop over batches ----------------
            denall = cpool.tile([1, BL * NQROW], F32, tag="denall")
            dscr = cpool.tile([1, NQROW], F32, tag="dscr")
            attn_sb = cpool.tile([128, BL * NQROW], F16, tag="attn_sb")

            kvpool = ctx.enter_context(tc.tile_pool(name="kv", bufs=8))
            expool = ctx.enter_context(tc.tile_pool(name="expp", bufs=3))
            attn_ctx = ExitStack()
            lps = attn_ctx.enter_context(tc.tile_pool(name="lps", bufs=4, space="PSUM"))
            atps = attn_ctx.enter_context(tc.tile_pool(name="atps", bufs=1, space="PSUM"))
            dnps = attn_ctx.enter_context(tc.tile_pool(name="dnps", bufs=2, space="PSUM"))

            attn_ps = atps.tile([128, BL * NQROW], F32, tag="attnp")

            kv_tiles = []
            for b in range(BL):
                k16 = kvpool.tile([128, n_tiles * K], F16, tag="k16")
                v16 = kvpool.tile([128, n_tiles * K], F16, tag="v16")
                nc.sync.dma_start(k16[:], ext["kc"].ap()[b])
                nc.sync.dma_start(v16[:], ext["vc"].ap()[b])
                kv_tiles.append((k16, v16))

            for b in range(BL):
                k16, v16 = kv_tiles[b]
                lg = lps.tile([128, LCOLS], F32, tag="lg")
                # kc is host-packed transposed: k16[k, s] -- tile j is cols j*128..
                for j in range(n_tiles):
                    nc.tensor.matmul(lg[:, j * NQROW:(j + 1) * NQROW],
                                     k16[:, j * K:(j + 1) * K],
                                     qT[:, b * NQROW:(b + 1) * NQROW],
                                     skip_group_check=True)

                ex = expool.tile([128, LCOLS], F16, tag="ex")
                nc.scalar.activation(ex[:], lg[:], mybir.ActivationFunctionType.Exp,
                                     bias=c_neg1[:, 0:1], scale=float(SCALE))

                # denominator: ones-matmul partials + strided reduce + fresh part
                dn = dnps.tile([1, LCOLS], F32, tag="dn")
                nc.tensor.matmul(dn[:], ones16[:, 0:1], ex[:], skip_group_check=True)
                nc.vector.reduce_sum(dscr[:],
                                     dn[:].rearrange("p (j q) -> p q j", j=n_tiles),
                                     axis=mybir.AxisListType.X)
                nc.vector.tensor_tensor(denall[:, b * NQROW:(b + 1) * NQROW], dscr[:],
                                        freshden[:, b * NQROW:(b + 1) * NQROW],
                                        mybir.AluOpType.add)

                # A.V accumulation: fresh first, then cache tiles
                nc.tensor.matmul(attn_ps[:, b * NQROW:(b + 1) * NQROW],
                                 vfl_s(b),
                                 expfr[0:T, b * NQROW:(b + 1) * NQROW],
                                 start=True, stop=False, skip_group_check=True)
                for j in range(n_tiles):
                    nc.tensor.matmul(attn_ps[:, b * NQROW:(b + 1) * NQROW],
                                     v16[:, j * K:(j + 1) * K],
                                     ex[:, j * NQROW:(j + 1) * NQROW],
                                     start=False, stop=(j == n_tiles - 1),
                                     skip_group_check=True)

            # ---------------- Normalize + o_proj + output ---------------------
            recip = cpool.tile([1, BL * NQROW], F32, tag="recip")
            rbc = cpool.tile([128, BL * NQROW], F32, tag="rbc")
            nc.vector.reciprocal(recip[:], denall[:])
            bcps = attn_ctx.enter_context(tc.tile_pool(name="bcps", bufs=1, space="PSUM"))
            rbp = bcps.tile([128, BL * NQROW], F32, tag="rbp")
            nc.tensor.matmul(rbp[:], ones_row[:], recip[:])
            nc.scalar.copy(rbc[:], rbp[:])
            # normalize and permute (b r t) -> (r b t) so o_proj lhsT slices are contiguous
            nc.vector.tensor_tensor(
                attn_sb[:].rearrange("p (r b t) -> p b r t", r=R, b=BL),
                attn_ps[:].rearrange("p (b r t) -> p b r t", r=R, b=BL),
                rbc[:].rearrange("p (b r t) -> p b r t", r=R, b=BL),
                mybir.AluOpType.mult)
            attn_ctx.close()

            out_sb = cpool.tile([TOK, D], F32, tag="out_sb")
            with tc.tile_pool(name="ps4", bufs=1, space="PSUM") as ps4, \
                 tc.tile_pool(name="dram", bufs=1, space="DRAM") as dpool:
                outp = ps4.tile([TOK, D], F32, tag="outp")
                if use_collective:
                    bin_ = dpool.tile([TOK, D], F32, tag="bin")
                    bout = dpool.tile([TOK, D], F32, tag="bout")
                # pipelined per-bank: matmuls -> copy -> bounce DMA overlap
                for n in range(D // 512):
                    cs = slice(n * 512, (n + 1) * 512)
                    for r in range(R):
                        nc.tensor.matmul(outp[:, cs],
                                         attn_sb[:, r * TOK:(r + 1) * TOK],
                                         wo[:, r * D + n * 512: r * D + (n + 1) * 512],
                                         start=(r == 0), stop=(r == R - 1))
                    if n % 2 == 0:
                        nc.vector.tensor_copy(out_sb[:, cs], outp[:, cs])
                    else:
                        nc.scalar.copy(out_sb[:, cs], outp[:, cs])
                    if use_collective:
                        nc.sync.dma_start(bin_[:, cs], out_sb[:, cs])
                    else:
                        nc.sync.dma_start(out_ext.ap()[:, cs], out_sb[:, cs])
                if use_collective:
                    nc.gpsimd.collective_compute(
                        "AllReduce",
                        mybir.AluOpType.add,
                        replica_groups=[[0, 1, 2, 3], [4, 5, 6, 7]],
                        ins=[bin_.opt()],
                        outs=[bout.opt()],
                    )
                    nc.sync.dma_start(out_ext.ap(), bout[:])

    nc.compile()
    return nc


def _prepare_inputs(hidden_BTD, segment_ids_BT, k_cache, v_cache, Wq, Wk, Wv, Wo,
                    q_scale, k_scale, cur):
    """Host-side sharding/packing. Returns (in_maps, n_tiles)."""
    hidden = np.asarray(hidden_BTD, np.float32)
    seg = np.asarray(segment_ids_BT)
    kc = np.asarray(k_cache, np.float32)
    vc = np.asarray(v_cache, np.float32)
    Wq = np.asarray(Wq, np.float32)
    Wk = np.asarray(Wk, np.float32)
    Wv = np.asarray(Wv, np.float32)
    Wo = np.asarray(Wo, np.float32)
    q_scale = np.asarray(q_scale, np.float32)
    k_scale = np.asarray(k_scale, np.float32)

    assert cur % 128 == 0 and cur + T <= S, f"unsupported cur_ind {cur}"
    n_tiles = cur // 128

    # positions / pads, exactly as the reference
    valid = (seg != 0)
    csum = np.cumsum(valid.astype(np.int32), axis=-1)
    left_pads = np.sum((csum == 0).astype(np.int32), axis=-1)
    assert np.all(left_pads == 0) and np.all(seg == 1), "only dense segments supported"
    positions = (csum - 1).astype(np.float32) + np.float32(cur)    # (B,T)

    # rope sin/cos in fp32 as the reference computes them
    fraction = np.arange(0, K // 2, dtype=np.float32) * np.float32(2.0 / K)
    timescale = (np.float32(ROPE_BASE) ** fraction).astype(np.float32)
    sinusoid = positions[..., None] / timescale                     # (B,T,64)
    sin = np.sin(sinusoid).astype(np.float32)
    cos = np.cos(sinusoid).astype(np.float32)

    def rope_tables(scale_vec):
        # A[i]: coefficient of x[i]; B[i]: coefficient of x[swap(i)]
        A = np.concatenate([cos * scale_vec[:K // 2], cos * scale_vec[K // 2:]], axis=-1)
        Bc = np.concatenate([-sin * scale_vec[K // 2:], sin * scale_vec[:K // 2]], axis=-1)
        return A.astype(np.float32), Bc.astype(np.float32)          # (B,T,128)

    Aq, Bq = rope_tables(q_scale)
    Ak, Bk = rope_tables(k_scale)

    # fresh-token multiplicative causal mask: (t', b*16 + r*4 + t) -> t' <= t
    m = (np.arange(T)[:, None] <= np.arange(T)[None, :]).astype(np.float16)  # (t',t)
    maskf = np.tile(m[:, None, None, :], (1, BL, R, 1)).reshape(T, BL * R * T)

    in_maps = []
    for c in range(NCORES):
        bh, g = c // 4, c % 4
        bsl = slice(bh * BL, (bh + 1) * BL)
        hT = hidden[bsl].reshape(TOK, D).T                          # (2048, 64)
        qcols = slice(g * QCOLS, (g + 1) * QCOLS)
        kcols = slice(g * K, (g + 1) * K)
        kloc = kc[bsl, :cur, g, :]                                  # (16, cur, 128)
        vloc = vc[bsl, :cur, g, :]
        # K is packed TRANSPOSED (k on partitions): kpack[b, k, s]
        kpack = np.ascontiguousarray(kloc.transpose(0, 2, 1)).astype(np.float16)
        vpack = np.ascontiguousarray(
            vloc.reshape(BL, n_tiles, 128, K).transpose(0, 2, 1, 3).reshape(BL, 128, n_tiles * K)).astype(np.float16)
        aq_l = np.tile(Aq[bsl].reshape(TOK, K), (1, R))
        bq_l = np.tile(Bq[bsl].reshape(TOK, K), (1, R))
        in_maps.append({
            "hT": _pack_rows(np.ascontiguousarray(hT)).astype(np.float16),
            "wq": _pack_rows(np.ascontiguousarray(Wq[:, qcols])).astype(np.float16),
            "wk": _pack_rows(np.ascontiguousarray(Wk[:, kcols])).astype(np.float16),
            "wv": _pack_rows(np.ascontiguousarray(Wv[:, kcols])).astype(np.float16),
            "wo": _pack_rows(np.ascontiguousarray(Wo[g * QCOLS:(g + 1) * QCOLS, :])).astype(np.float16),
            "kc": kpack,
            "vc": vpack,
            "aq": np.ascontiguousarray(aq_l),
            "bq": np.ascontiguousarray(bq_l),
            "ak": np.ascontiguousarray(Ak[bsl].reshape(TOK, K)),
            "bk": np.ascontiguousarray(Bk[bsl].reshape(TOK, K)),
            "ident16": np.eye(128, dtype=np.float16),
            "ones16": np.ones((128, 1), np.float16),
            "maskf": maskf,
        })
    return in_maps, n_tiles


def kernel(**inputs):
    cur = int(np.asarray(inputs["cur_ind"]))
    in_maps, n_tiles = _prepare_inputs(
        inputs["hidden_BTD"], inputs["segment_ids_BT"], inputs["k_cache"],
        inputs["v_cache"], inputs["Wq"], inputs["Wk"], inputs["Wv"], inputs["Wo"],
        inputs["q_scale"], inputs["k_scale"], cur)

    for use_coll in (True, False):
        key = (cur, use_coll)
        try:
            if key not in _COMPILED:
                _COMPILED[key] = _build_nc(cur, n_tiles, use_coll)
            nc = _COMPILED[key]
            res = run_bass_kernel_spmd(nc, in_maps, list(range(NCORES)))
            outs = [res.results[c]["out"].reshape(BL, T, D) for c in range(NCORES)]
            if use_coll:
                full = np.concatenate([outs[0], outs[4]], axis=0)
            else:
                full = np.concatenate([sum(outs[0:4]), sum(outs[4:8])], axis=0)
            return full.astype(np.float32)
        except Exception:
            if not use_coll:
                raise
            import traceback
            traceback.print_exc()
            print("collective path failed; falling back to host-side reduce",
                  file=sys.stderr)
    raise RuntimeError("unreachable")



# revision 4
# speedup vs baseline: 382.4857x; 382.4857x over previous
"""Trainium2 Bass kernel for GQA decode attention (B=32,T=4,D=2048,H=16,G=4,K=128,S=4096).

Sharding: 8 NeuronCores = 2 batch-groups x 4 kv-head groups.
Core c: batches [16*(c//4), 16*(c//4)+16), kv head g = c % 4 (owns 4 q heads).
o_proj partial sums are AllReduce'd across each 4-core head group on device.

Device pipeline per core:
  - QKV projection (fp16 matmuls, fp32 PSUM accumulate)
  - RMS-norm + RoPE in fp32 on DVE/ACT (host-precomputed coefficient tables
    with q_scale/k_scale folded in)
  - attention in transposed orientation: PE-transpose K tiles, logits^T =
    kT @ qT (fp16 in, fp32 out), softmax WITHOUT max-subtraction -- safe
    because rms-normed q,k bound |logits| <= sqrt(K); exp computes
    exp(x/sqrt(K) - 1) on ACT (the -1 cancels in normalization and keeps
    fp16 exp values < 65504)
  - A.V accumulates attn^T directly (v natural layout as the stationary)
  - fp16 o_proj into fp32 PSUM, AllReduce, DMA out

Only cache rows [0, cur_ind) are read; rows [cur_ind, cur_ind+T) are the
freshly projected k/v handled on-chip, rows beyond are masked by the
reference -- so the cache update never materializes.
"""

import sys

sys.path.insert(0, "/opt/trn_rl_repo")

import numpy as np

import concourse.bacc as bacc
import concourse.mybir as mybir
import concourse.tile as tile
from concourse.bass_utils import run_bass_kernel_spmd

F32 = mybir.dt.float32
F32R = mybir.dt.float32r
F16 = mybir.dt.float16

B, T, D = 32, 4, 2048
H, G, K = 16, 4, 128
S = 4096
R = H // G          # 4 q heads per kv head
EPS = 1e-6
ROPE_BASE = 10000.0
NCORES = 8
BG = 2              # batch groups
BL = B // BG        # 16 batches per core
TOK = BL * T        # 64 tokens per core
QCOLS = R * K       # 512 local q columns
NDC = D // 128      # 16 contraction chunks for qkv proj
SCALE = 1.0 / np.sqrt(np.float32(K))
EXP_BIAS = -1.0     # exp(x*SCALE + EXP_BIAS); cancels in softmax, avoids fp16 overflow

_COMPILED = {}


def _pack_rows(w):
    """(C*128, N) -> (128, C*N) with [p, c*N+n] = w[c*128+p, n]."""
    c = w.shape[0] // 128
    n = w.shape[1]
    return np.ascontiguousarray(
        w.reshape(c, 128, n).transpose(1, 0, 2).reshape(128, c * n)
    )


def _build_nc(cur, n_tiles, use_collective, nreps=1):
    """Build the kernel program. nreps>1 emits the complete body (all DRAM
    loads + compute + output) back-to-back that many times in one program —
    used by the benchmark harness to measure steady-state per-invocation HW
    time with dispatch overhead amortized. Every rep re-reads all inputs from
    DRAM and rewrites the output, so per-rep work is identical to nreps=1."""
    nc = bacc.Bacc("TRN2", target_bir_lowering=False, debug=False, num_devices=NCORES)

    ext = {}

    def inp(name, shape, dt=F32):
        ext[name] = nc.dram_tensor(name, list(shape), dt, kind="ExternalInput")
        return ext[name]

    inp("hT", (128, NDC * TOK), F16)       # hiddenT packed per d-chunk
    inp("wq", (128, NDC * QCOLS), F16)
    inp("wk", (128, NDC * K), F16)
    inp("wv", (128, NDC * K), F16)
    inp("wo", (128, R * D), F16)
    inp("kc", (BL, 128, n_tiles * K), F16) # cache, host-packed (p = s%128), fp16
    inp("vc", (BL, 128, n_tiles * K), F16)
    inp("aq", (TOK, QCOLS))
    inp("bq", (TOK, QCOLS))
    inp("ak", (TOK, K))
    inp("bk", (TOK, K))
    inp("ident16", (128, 128), F16)
    inp("ones16", (128, 1), F16)
    inp("maskf", (T, BL * 4 * T), F16)     # multiplicative causal mask for fresh tokens
    out_ext = nc.dram_tensor("out", [TOK, D], F32, kind="ExternalOutput")

    NQROW = 4 * T                          # 16 query rows per batch (r*4+t)
    LCOLS = n_tiles * NQROW                # logitsT bank cols per batch

    with tile.TileContext(nc) as tc:
        from contextlib import ExitStack

        for _rep in range(nreps):
          with ExitStack() as ctx:
            cpool = ctx.enter_context(tc.tile_pool(name="const", bufs=1))

            def load(name, dt=None, eng=None, split=1):
                h = ext[name]
                t_ = cpool.tile(list(h.shape), dt or h.dtype, tag=name)
                ncols = h.shape[-1]
                step = ncols // split
                for s0 in range(0, ncols, step):
                    (eng or nc.sync).dma_start(t_[:, s0:s0 + step], h.ap()[:, s0:s0 + step])
                return t_

            hT = load("hT", split=4)
            wq = load("wq", split=4)
            wk = load("wk", split=4)
            wv = load("wv", split=4)
            wo = load("wo", split=2)
            aq = load("aq")
            bq = load("bq")
            ak = load("ak")
            bk = load("bk")
            ident16 = load("ident16")
            ones16 = load("ones16")
            maskf = load("maskf")

            # ---------------- Phase 1: QKV projection + norm + rope ----------
            c_eps = cpool.tile([128, 1], F32, tag="c_eps")
            c_neg1 = cpool.tile([128, 1], F32, tag="c_neg1")
            nc.vector.memset(c_eps[:], float(EPS))
            nc.vector.memset(c_neg1[:], float(EXP_BIAS))
            ones_row = cpool.tile([1, 128], F32, tag="ones_row")
            nc.vector.memset(ones_row[:], 1.0)

            qn = cpool.tile([TOK, QCOLS], F32, tag="qn")       # normed+roped q
            kn = cpool.tile([TOK, K], F32, tag="kn")
            WQKV = QCOLS + 2 * K                               # 768 combined cols
            qkv16 = cpool.tile([TOK, WQKV], F16, tag="qkv16")
            q16 = qkv16[:, 0:QCOLS]
            k16n = qkv16[:, QCOLS:QCOLS + K]
            v16n = qkv16[:, QCOLS + K:WQKV]
            ssq = cpool.tile([TOK, 8], F32, tag="ssq")
            rstd = cpool.tile([TOK, 8], F32, tag="rstd")
            scr = cpool.tile([TOK, QCOLS], F32, tag="scr")
            scr2 = cpool.tile([TOK, QCOLS], F32, tag="scr2")

            with tc.tile_pool(name="ps1", bufs=1, space="PSUM") as ps1:
                pq = ps1.tile([TOK, QCOLS], F32, tag="pq")
                pk = ps1.tile([TOK, K], F32, tag="pk")
                pv = ps1.tile([TOK, K], F32, tag="pv")
                for c in range(NDC):
                    lhs = hT[:, c * TOK:(c + 1) * TOK]
                    st, sp = (c == 0), (c == NDC - 1)
                    nc.tensor.matmul(pq[:], lhs, wq[:, c * QCOLS:(c + 1) * QCOLS], start=st, stop=sp)
                    nc.tensor.matmul(pk[:], lhs, wk[:, c * K:(c + 1) * K], start=st, stop=sp)
                    nc.tensor.matmul(pv[:], lhs, wv[:, c * K:(c + 1) * K], start=st, stop=sp)

                # sum of squares per (token, head)
                for h in range(R):
                    nc.scalar.activation(scr[:, h * K:(h + 1) * K], pq[:, h * K:(h + 1) * K],
                                         mybir.ActivationFunctionType.Square,
                                         accum_out=ssq[:, h:h + 1])
                nc.scalar.activation(scr2[:, 0:K], pk[:],
                                     mybir.ActivationFunctionType.Square,
                                     accum_out=ssq[:, R:R + 1])
                # std = sqrt(ssq/K + eps); rstd = 1/std
                nc.scalar.activation(rstd[:, 0:5], ssq[:, 0:5],
                                     mybir.ActivationFunctionType.Sqrt,
                                     bias=c_eps[0:TOK, 0:1], scale=float(1.0 / K))
                nc.vector.reciprocal(ssq[:, 0:5], rstd[:, 0:5])
                # q_hat = q * rstd (per token/head), same for k
                for h in range(R):
                    nc.vector.tensor_scalar(qn[:, h * K:(h + 1) * K], pq[:, h * K:(h + 1) * K],
                                            ssq[:, h:h + 1], None, mybir.AluOpType.mult)
                nc.vector.tensor_scalar(kn[:], pk[:], ssq[:, R:R + 1], None, mybir.AluOpType.mult)
                # v -> fp16
                nc.scalar.copy(v16n, pv[:])

            # rope: out = qh*A + swap_halves(qh)*B   (scale folded into A/B)
            def rope(dst16, x, a, b, s1, s2, nh):
                xr = x[:].rearrange("p (h u x) -> p h u x", h=nh, u=2)
                # dst16 is an AP slice of the combined qkv16 tile
                br = b[:].rearrange("p (h u x) -> p h u x", h=nh, u=2)
                s2r = s2[:, 0:nh * K].rearrange("p (h u x) -> p h u x", h=nh, u=2)
                nc.vector.tensor_tensor(s1[:, 0:nh * K], x[:, 0:nh * K], a[:, 0:nh * K], mybir.AluOpType.mult)
                # swapped-half products
                nc.vector.tensor_tensor(s2r[:, :, 0, :], xr[:, :, 1, :], br[:, :, 0, :], mybir.AluOpType.mult)
                nc.vector.tensor_tensor(s2r[:, :, 1, :], xr[:, :, 0, :], br[:, :, 1, :], mybir.AluOpType.mult)
                nc.vector.tensor_tensor(s1[:, 0:nh * K], s1[:, 0:nh * K], s2[:, 0:nh * K], mybir.AluOpType.add)
                nc.vector.tensor_copy(dst16, s1[:, 0:nh * K])

            rope(q16, qn, aq, bq, scr, scr2, R)
            rope(k16n, kn, ak, bk, scr, scr2, 1)

            # flatten only v's (b t) partition layout -> t-partitions (base 0)
            vfl = cpool.tile([T, BL * K], F16, tag="vfl")
            for t in range(T):
                nc.gpsimd.dma_start(vfl[t:t + 1, :], v16n[t::T, :])

            def vfl_s(b):
                return vfl[0:T, b * K:(b + 1) * K]

            # transpose q -> qT (128k x 16 qrow per b), k_new -> kTnew (128k x 4 per b)
            # qT via 4 whole-column transposes (64 tokens at once, base-0 aligned),
            # then one copy permuting (r b t) -> (b r t); kTnew via one transpose.
            qT = cpool.tile([128, BL * NQROW], F16, tag="qT")
            kTnew = cpool.tile([128, BL * T], F16, tag="kTnew")
            with tc.tile_pool(name="ps2", bufs=1, space="PSUM") as ps2:
                qTp = ps2.tile([128, BL * NQROW], F16, tag="qTp")
                kTnp = ps2.tile([128, BL * T], F16, tag="kTnp")
                for r in range(R):
                    nc.tensor.matmul(qTp[:, r * TOK:(r + 1) * TOK],
                                     q16[:, r * K:(r + 1) * K],
                                     ident16[0:TOK, 0:TOK], is_transpose=True,
                                     skip_group_check=True)
                nc.tensor.matmul(kTnp[:], k16n,
                                 ident16[0:TOK, 0:TOK], is_transpose=True,
                                 skip_group_check=True)
                nc.scalar.copy(
                    qT[:].rearrange("p (b r t) -> p b r t", b=BL, r=R),
                    qTp[:].rearrange("p (r b t) -> p b r t", r=R, b=BL))
                nc.scalar.copy(kTnew[:], kTnp[:])

            # ---------------- Fresh-token logits / exp / denom ---------------
            expfr = cpool.tile([T, BL * NQROW], F16, tag="expfr")
            freshden = cpool.tile([1, BL * NQROW], F32, tag="freshden")
            with tc.tile_pool(name="ps3", bufs=1, space="PSUM") as ps3:
                frp = ps3.tile([T, BL * NQROW], F32, tag="frp")
                fdp = ps3.tile([1, BL * NQROW], F32, tag="fdp")
                for b in range(BL):
                    nc.tensor.matmul(frp[0:T, b * NQROW:(b + 1) * NQROW],
                                     kTnew[:, b * T:(b + 1) * T],
                                     qT[:, b * NQROW:(b + 1) * NQROW],
                                     skip_group_check=True)
                nc.scalar.activation(expfr[:], frp[:], mybir.ActivationFunctionType.Exp,
                                     bias=c_neg1[0:T, 0:1], scale=float(SCALE))
                nc.vector.tensor_tensor(expfr[:], expfr[:], maskf[:], mybir.AluOpType.mult)
                nc.tensor.matmul(fdp[:], ones16[0:T, 0:1], expfr[:])
                nc.scalar.copy(freshden[:], fdp[:])

            # ---------------- Main attention loop over batches ----------------
            denall = cpool.tile([1, BL * NQROW], F32, tag="denall")
            dscr = cpool.tile([1, NQROW], F32, tag="dscr")
            attn_sb = cpool.tile([128, BL * NQROW], F16, tag="attn_sb")

            kvpool = ctx.enter_context(tc.tile_pool(name="kv", bufs=8))
            expool = ctx.enter_context(tc.tile_pool(name="expp", bufs=3))
            attn_ctx = ExitStack()
            lps = attn_ctx.enter_context(tc.tile_pool(name="lps", bufs=4, space="PSUM"))
            atps = attn_ctx.enter_context(tc.tile_pool(name="atps", bufs=1, space="PSUM"))
            dnps = attn_ctx.enter_context(tc.tile_pool(name="dnps", bufs=2, space="PSUM"))

            attn_ps = atps.tile([128, BL * NQROW], F32, tag="attnp")

            kv_tiles = []
            for b in range(BL):
                k16 = kvpool.tile([128, n_tiles * K], F16, tag="k16")
                v16 = kvpool.tile([128, n_tiles * K], F16, tag="v16")
                nc.sync.dma_start(k16[:], ext["kc"].ap()[b])
                nc.sync.dma_start(v16[:], ext["vc"].ap()[b])
                kv_tiles.append((k16, v16))

            for b in range(BL):
                k16, v16 = kv_tiles[b]
                lg = lps.tile([128, LCOLS], F32, tag="lg")
                # kc is host-packed transposed: k16[k, s] -- tile j is cols j*128..
                for j in range(n_tiles):
                    nc.tensor.matmul(lg[:, j * NQROW:(j + 1) * NQROW],
                                     k16[:, j * K:(j + 1) * K],
                                     qT[:, b * NQROW:(b + 1) * NQROW],
                                     skip_group_check=True)

                ex = expool.tile([128, LCOLS], F16, tag="ex")
                nc.scalar.activation(ex[:], lg[:], mybir.ActivationFunctionType.Exp,
                                     bias=c_neg1[:, 0:1], scale=float(SCALE))

                # denominator: ones-matmul partials + strided reduce + fresh part
                dn = dnps.tile([1, LCOLS], F32, tag="dn")
                nc.tensor.matmul(dn[:], ones16[:, 0:1], ex[:], skip_group_check=True)
                nc.vector.reduce_sum(dscr[:],
                                     dn[:].rearrange("p (j q) -> p q j", j=n_tiles),
                                     axis=mybir.AxisListType.X)
                nc.vector.tensor_tensor(denall[:, b * NQROW:(b + 1) * NQROW], dscr[:],
                                        freshden[:, b * NQROW:(b + 1) * NQROW],
                                        mybir.AluOpType.add)

                # A.V accumulation: fresh first, then cache tiles
                nc.tensor.matmul(attn_ps[:, b * NQROW:(b + 1) * NQROW],
                                 vfl_s(b),
                                 expfr[0:T, b * NQROW:(b + 1) * NQROW],
                                 start=True, stop=False, skip_group_check=True)
                for j in range(n_tiles):
                    nc.tensor.matmul(attn_ps[:, b * NQROW:(b + 1) * NQROW],
                                     v16[:, j * K:(j + 1) * K],
                                     ex[:, j * NQROW:(j + 1) * NQROW],
                                     start=False, stop=(j == n_tiles - 1),
                                     skip_group_check=True)

            # ---------------- Normalize + o_proj + output ---------------------
            recip = cpool.tile([1, BL * NQROW], F32, tag="recip")
            rbc = cpool.tile([128, BL * NQROW], F32, tag="rbc")
            nc.vector.reciprocal(recip[:], denall[:])
            bcps = attn_ctx.enter_context(tc.tile_pool(name="bcps", bufs=1, space="PSUM"))
            rbp = bcps.tile([128, BL * NQROW], F32, tag="rbp")
            nc.tensor.matmul(rbp[:], ones_row[:], recip[:])
            nc.scalar.copy(rbc[:], rbp[:])
            # normalize and permute (b r t) -> (r b t) so o_proj lhsT slices are contiguous
            nc.vector.tensor_tensor(
                attn_sb[:].rearrange("p (r b t) -> p b r t", r=R, b=BL),
                attn_ps[:].rearrange("p (b r t) -> p b r t", r=R, b=BL),
                rbc[:].rearrange("p (b r t) -> p b r t", r=R, b=BL),
                mybir.AluOpType.mult)
            attn_ctx.close()

            out_sb = cpool.tile([TOK, D], F32, tag="out_sb")
            with tc.tile_pool(name="ps4", bufs=1, space="PSUM") as ps4, \
                 tc.tile_pool(name="dram", bufs=1, space="DRAM") as dpool:
                outp = ps4.tile([TOK, D], F32, tag="outp")
                if use_collective:
                    bin_ = dpool.tile([TOK, D], F32, tag="bin")
                    bout = dpool.tile([TOK, D], F32, tag="bout")
                # pipelined per-bank: matmuls -> copy -> bounce DMA overlap
                for n in range(D // 512):
                    cs = slice(n * 512, (n + 1) * 512)
                    for r in range(R):
                        nc.tensor.matmul(outp[:, cs],
                                         attn_sb[:, r * TOK:(r + 1) * TOK],
                                         wo[:, r * D + n * 512: r * D + (n + 1) * 512],
                                         start=(r == 0), stop=(r == R - 1))
                    if n % 2 == 0:
                        nc.vector.tensor_copy(out_sb[:, cs], outp[:, cs])
                    else:
                        nc.scalar.copy(out_sb[:, cs], outp[:, cs])
                    if use_collective:
                        nc.sync.dma_start(bin_[:, cs], out_sb[:, cs])
                    else:
                        nc.sync.dma_start(out_ext.ap()[:, cs], out_sb[:, cs])
                if use_collective:
                    nc.gpsimd.collective_compute(
                        "AllReduce",
                        mybir.AluOpType.add,
                        replica_groups=[[0, 1, 2, 3], [4, 5, 6, 7]],
                        ins=[bin_.opt()],
                        outs=[bout.opt()],
                    )
                    nc.sync.dma_start(out_ext.ap(), bout[:])

    nc.compile()
    return nc


def _prepare_inputs(hidden_BTD, segment_ids_BT, k_cache, v_cache, Wq, Wk, Wv, Wo,
                    q_scale, k_scale, cur):
    """Host-side sharding/packing. Returns (in_maps, n_tiles)."""
    hidden = np.asarray(hidden_BTD, np.float32)
    seg = np.asarray(segment_ids_BT)
    kc = np.asarray(k_cache, np.float32)
    vc = np.asarray(v_cache, np.float32)
    Wq = np.asarray(Wq, np.float32)
    Wk = np.asarray(Wk, np.float32)
    Wv = np.asarray(Wv, np.float32)
    Wo = np.asarray(Wo, np.float32)
    q_scale = np.asarray(q_scale, np.float32)
    k_scale = np.asarray(k_scale, np.float32)

    assert cur % 128 == 0 and cur + T <= S, f"unsupported cur_ind {cur}"
    n_tiles = cur // 128

    # positions / pads, exactly as the reference
    valid = (seg != 0)
    csum = np.cumsum(valid.astype(np.int32), axis=-1)
    left_pads = np.sum((csum == 0).astype(np.int32), axis=-1)
    assert np.all(left_pads == 0) and np.all(seg == 1), "only dense segments supported"
    positions = (csum - 1).astype(np.float32) + np.float32(cur)    # (B,T)

    # rope sin/cos in fp32 as the reference computes them
    fraction = np.arange(0, K // 2, dtype=np.float32) * np.float32(2.0 / K)
    timescale = (np.float32(ROPE_BASE) ** fraction).astype(np.float32)
    sinusoid = positions[..., None] / timescale                     # (B,T,64)
    sin = np.sin(sinusoid).astype(np.float32)
    cos = np.cos(sinusoid).astype(np.float32)

    def rope_tables(scale_vec):
        # A[i]: coefficient of x[i]; B[i]: coefficient of x[swap(i)]
        A = np.concatenate([cos * scale_vec[:K // 2], cos * scale_vec[K // 2:]], axis=-1)
        Bc = np.concatenate([-sin * scale_vec[K // 2:], sin * scale_vec[:K // 2]], axis=-1)
        return A.astype(np.float32), Bc.astype(np.float32)          # (B,T,128)

    Aq, Bq = rope_tables(q_scale)
    Ak, Bk = rope_tables(k_scale)

    # fresh-token multiplicative causal mask: (t', b*16 + r*4 + t) -> t' <= t
    m = (np.arange(T)[:, None] <= np.arange(T)[None, :]).astype(np.float16)  # (t',t)
    maskf = np.tile(m[:, None, None, :], (1, BL, R, 1)).reshape(T, BL * R * T)

    in_maps = []
    for c in range(NCORES):
        bh, g = c // 4, c % 4
        bsl = slice(bh * BL, (bh + 1) * BL)
        hT = hidden[bsl].reshape(TOK, D).T                          # (2048, 64)
        qcols = slice(g * QCOLS, (g + 1) * QCOLS)
        kcols = slice(g * K, (g + 1) * K)
        kloc = kc[bsl, :cur, g, :]                                  # (16, cur, 128)
        vloc = vc[bsl, :cur, g, :]
        # K is packed TRANSPOSED (k on partitions): kpack[b, k, s]
        kpack = np.ascontiguousarray(kloc.transpose(0, 2, 1)).astype(np.float16)
        vpack = np.ascontiguousarray(
            vloc.reshape(BL, n_tiles, 128, K).transpose(0, 2, 1, 3).reshape(BL, 128, n_tiles * K)).astype(np.float16)
        aq_l = np.tile(Aq[bsl].reshape(TOK, K), (1, R))
        bq_l = np.tile(Bq[bsl].reshape(TOK, K), (1, R))
        in_maps.append({
            "hT": _pack_rows(np.ascontiguousarray(hT)).astype(np.float16),
            "wq": _pack_rows(np.ascontiguousarray(Wq[:, qcols])).astype(np.float16),
            "wk": _pack_rows(np.ascontiguousarray(Wk[:, kcols])).astype(np.float16),
            "wv": _pack_rows(np.ascontiguousarray(Wv[:, kcols])).astype(np.float16),
            "wo": _pack_rows(np.ascontiguousarray(Wo[g * QCOLS:(g + 1) * QCOLS, :])).astype(np.float16),
            "kc": kpack,
            "vc": vpack,
            "aq": np.ascontiguousarray(aq_l),
            "bq": np.ascontiguousarray(bq_l),
            "ak": np.ascontiguousarray(Ak[bsl].reshape(TOK, K)),
            "bk": np.ascontiguousarray(Bk[bsl].reshape(TOK, K)),
            "ident16": np.eye(128, dtype=np.float16),
            "ones16": np.ones((128, 1), np.float16),
            "maskf": maskf,
        })
    return in_maps, n_tiles


def kernel(**inputs):
    cur = int(np.asarray(inputs["cur_ind"]))
    in_maps, n_tiles = _prepare_inputs(
        inputs["hidden_BTD"], inputs["segment_ids_BT"], inputs["k_cache"],
        inputs["v_cache"], inputs["Wq"], inputs["Wk"], inputs["Wv"], inputs["Wo"],
        inputs["q_scale"], inputs["k_scale"], cur)

    for use_coll in (True, False):
        key = (cur, use_coll, 1)
        try:
            if key not in _COMPILED:
                _COMPILED[key] = _build_nc(cur, n_tiles, use_coll, nreps=1)
            nc = _COMPILED[key]
            res = run_bass_kernel_spmd(nc, in_maps, list(range(NCORES)))
            outs = [res.results[c]["out"].reshape(BL, T, D) for c in range(NCORES)]
            if use_coll:
                full = np.concatenate([outs[0], outs[4]], axis=0)
            else:
                full = np.concatenate([sum(outs[0:4]), sum(outs[4:8])], axis=0)
            return full.astype(np.float32)
        except Exception:
            if not use_coll:
                raise
            import traceback
            traceback.print_exc()
            print("collective path failed; falling back to host-side reduce",
                  file=sys.stderr)
    raise RuntimeError("unreachable")



# revision 12
# speedup vs baseline: 504.4158x; 1.3188x over previous
"""Trainium2 Bass kernel for GQA decode attention (B=32,T=4,D=2048,H=16,G=4,K=128,S=4096).

Sharding: 8 NeuronCores = 2 batch-groups x 4 kv-head groups.
Core c: batches [16*(c//4), 16*(c//4)+16), kv head g = c % 4 (owns 4 q heads).
o_proj partial sums are AllReduce'd across each 4-core head group on device.

Device pipeline per core:
  - QKV projection (fp16 matmuls, fp32 PSUM accumulate)
  - RMS-norm + RoPE in fp32 on DVE/ACT (host-precomputed coefficient tables
    with q_scale/k_scale folded in)
  - attention in transposed orientation: PE-transpose K tiles, logits^T =
    kT @ qT (fp16 in, fp32 out), softmax WITHOUT max-subtraction -- safe
    because rms-normed q,k bound |logits| <= sqrt(K); exp computes
    exp(x/sqrt(K) - 1) on ACT (the -1 cancels in normalization and keeps
    fp16 exp values < 65504)
  - A.V accumulates attn^T directly (v natural layout as the stationary)
  - fp16 o_proj into fp32 PSUM, AllReduce, DMA out

Only cache rows [0, cur_ind) are read; rows [cur_ind, cur_ind+T) are the
freshly projected k/v handled on-chip, rows beyond are masked by the
reference -- so the cache update never materializes.
"""

import sys

sys.path.insert(0, "/opt/trn_rl_repo")

import numpy as np
import ml_dtypes

F8NP = ml_dtypes.float8_e3m4
WSCALE = 32.0

import concourse.bacc as bacc
import concourse.mybir as mybir
import concourse.tile as tile
from concourse.bass_utils import run_bass_kernel_spmd

F32 = mybir.dt.float32
F32R = mybir.dt.float32r
F16 = mybir.dt.float16
F8 = mybir.dt.float8e3   # TRN E3M4: ~1.6% normal / 2^-6 denormal step
# WSCALE (defined above): prescale for fp8 qkv weights; rms-norm cancels it
# for q/k, the fresh-v copy divides it back out

B, T, D = 32, 4, 2048
H, G, K = 16, 4, 128
S = 4096
R = H // G          # 4 q heads per kv head
EPS = 1e-6
ROPE_BASE = 10000.0
NCORES = 8
BG = 2              # batch groups
BL = B // BG        # 16 batches per core
TOK = BL * T        # 64 tokens per core
QCOLS = R * K       # 512 local q columns
NDC = D // 128      # 16 contraction chunks for qkv proj
SCALE = 1.0 / np.sqrt(np.float32(K))
EXP_BIAS = -1.0     # exp(x*SCALE + EXP_BIAS); cancels in softmax, avoids fp16 overflow

_COMPILED = {}


def _pack_rows(w):
    """(C*128, N) -> (128, C*N) with [p, c*N+n] = w[c*128+p, n]."""
    c = w.shape[0] // 128
    n = w.shape[1]
    return np.ascontiguousarray(
        w.reshape(c, 128, n).transpose(1, 0, 2).reshape(128, c * n)
    )


def _build_nc(cur, n_tiles, use_collective, nreps=1):
    """Build the kernel program. nreps>1 emits the complete body (all DRAM
    loads + compute + output) back-to-back that many times in one program —
    used by the benchmark harness to measure steady-state per-invocation HW
    time with dispatch overhead amortized. Every rep re-reads all inputs from
    DRAM and rewrites the output, so per-rep work is identical to nreps=1."""
    nc = bacc.Bacc("TRN2", target_bir_lowering=False, debug=False, num_devices=NCORES)

    ext = {}

    def inp(name, shape, dt=F32):
        ext[name] = nc.dram_tensor(name, list(shape), dt, kind="ExternalInput")
        return ext[name]

    inp("hT", (128, NDC * TOK), F16)       # hiddenT packed per d-chunk
    inp("wq", (128, NDC * QCOLS), F8)      # qkv weights fp8, prescaled by WSCALE
    inp("wk", (128, NDC * K), F8)
    inp("wv", (128, NDC * K), F8)
    inp("wo", (128, R * D), F16)
    inp("kc", (BL, 128, n_tiles * K), F8)  # cache K, host-packed (p = s%128), fp8
    inp("vc", (BL, 128, n_tiles * K), F16) # cache V stays fp16 (direct output error)
    inp("aq", (TOK, QCOLS))
    inp("bq", (TOK, QCOLS))
    inp("ak", (TOK, K))
    inp("bk", (TOK, K))
    inp("ident16", (128, 128), F16)
    inp("ones16", (128, 1), F16)
    inp("maskf", (T, BL * 4 * T), F16)     # multiplicative causal mask for fresh tokens
    out_ext = nc.dram_tensor("out", [TOK, D], F32, kind="ExternalOutput")

    NQROW = 4 * T                          # 16 query rows per batch (r*4+t)
    LCOLS = n_tiles * NQROW                # logitsT bank cols per batch

    with tile.TileContext(nc) as tc:
        from contextlib import ExitStack

        for _rep in range(nreps):
          with ExitStack() as ctx:
            cpool = ctx.enter_context(tc.tile_pool(name="const", bufs=1))

            def load(name, dt=None, eng=None, split=1):
                h = ext[name]
                t_ = cpool.tile(list(h.shape), dt or h.dtype, tag=name)
                ncols = h.shape[-1]
                step = ncols // split
                for s0 in range(0, ncols, step):
                    (eng or nc.sync).dma_start(t_[:, s0:s0 + step], h.ap()[:, s0:s0 + step])
                return t_

            hT = load("hT", split=4)
            wq = load("wq", split=4)
            wk = load("wk", split=4)
            wv = load("wv", split=4)
            wo = load("wo", split=2)
            aq = load("aq")
            bq = load("bq")
            ak = load("ak")
            bk = load("bk")
            ident16 = load("ident16")
            ones16 = load("ones16")
            maskf = load("maskf")

            # ---------------- Phase 1: QKV projection + norm + rope ----------
            c_eps = cpool.tile([128, 1], F32, tag="c_eps")
            c_neg1 = cpool.tile([128, 1], F32, tag="c_neg1")
            nc.vector.memset(c_eps[:], float(EPS))
            nc.vector.memset(c_neg1[:], float(EXP_BIAS))
            ones_row = cpool.tile([1, 128], F32, tag="ones_row")
            nc.vector.memset(ones_row[:], 1.0)

            qn = cpool.tile([TOK, QCOLS], F32, tag="qn")       # normed+roped q
            kn = cpool.tile([TOK, K], F32, tag="kn")
            WQKV = QCOLS + 2 * K                               # 768 combined cols
            qkv16 = cpool.tile([TOK, WQKV], F16, tag="qkv16")
            q16 = qkv16[:, 0:QCOLS]
            k16n = qkv16[:, QCOLS:QCOLS + K]
            v16n = qkv16[:, QCOLS + K:WQKV]
            ssq = cpool.tile([TOK, 8], F32, tag="ssq")
            rstd = cpool.tile([TOK, 8], F32, tag="rstd")
            scr = cpool.tile([TOK, QCOLS], F32, tag="scr")
            scr2 = cpool.tile([TOK, QCOLS], F32, tag="scr2")

            with tc.tile_pool(name="ps1", bufs=1, space="PSUM") as ps1:
                pq = ps1.tile([TOK, QCOLS], F32, tag="pq")
                pk = ps1.tile([TOK, K], F32, tag="pk")
                pv = ps1.tile([TOK, K], F32, tag="pv")
                for c in range(NDC):
                    lhs = hT[:, c * TOK:(c + 1) * TOK]
                    st, sp = (c == 0), (c == NDC - 1)
                    nc.tensor.matmul(pq[:], lhs, wq[:, c * QCOLS:(c + 1) * QCOLS], start=st, stop=sp)
                    nc.tensor.matmul(pk[:], lhs, wk[:, c * K:(c + 1) * K], start=st, stop=sp)
                    nc.tensor.matmul(pv[:], lhs, wv[:, c * K:(c + 1) * K], start=st, stop=sp)

                # sum of squares per (token, head)
                for h in range(R):
                    nc.scalar.activation(scr[:, h * K:(h + 1) * K], pq[:, h * K:(h + 1) * K],
                                         mybir.ActivationFunctionType.Square,
                                         accum_out=ssq[:, h:h + 1])
                nc.scalar.activation(scr2[:, 0:K], pk[:],
                                     mybir.ActivationFunctionType.Square,
                                     accum_out=ssq[:, R:R + 1])
                # std = sqrt(ssq/K + eps); rstd = 1/std
                nc.scalar.activation(rstd[:, 0:5], ssq[:, 0:5],
                                     mybir.ActivationFunctionType.Sqrt,
                                     bias=c_eps[0:TOK, 0:1], scale=float(1.0 / K))
                nc.vector.reciprocal(ssq[:, 0:5], rstd[:, 0:5])
                # q_hat = q * rstd (per token/head), same for k
                for h in range(R):
                    nc.vector.tensor_scalar(qn[:, h * K:(h + 1) * K], pq[:, h * K:(h + 1) * K],
                                            ssq[:, h:h + 1], None, mybir.AluOpType.mult)
                nc.vector.tensor_scalar(kn[:], pk[:], ssq[:, R:R + 1], None, mybir.AluOpType.mult)
                # v -> fp16, undoing the fp8 weight prescale
                nc.scalar.activation(v16n, pv[:],
                                     mybir.ActivationFunctionType.Copy,
                                     scale=float(1.0 / WSCALE))

            # rope: out = qh*A + swap_halves(qh)*B   (scale folded into A/B)
            def rope(dst16, x, a, b, s1, s2, nh):
                xr = x[:].rearrange("p (h u x) -> p h u x", h=nh, u=2)
                # dst16 is an AP slice of the combined qkv16 tile
                br = b[:].rearrange("p (h u x) -> p h u x", h=nh, u=2)
                s2r = s2[:, 0:nh * K].rearrange("p (h u x) -> p h u x", h=nh, u=2)
                nc.vector.tensor_tensor(s1[:, 0:nh * K], x[:, 0:nh * K], a[:, 0:nh * K], mybir.AluOpType.mult)
                # swapped-half products
                nc.vector.tensor_tensor(s2r[:, :, 0, :], xr[:, :, 1, :], br[:, :, 0, :], mybir.AluOpType.mult)
                nc.vector.tensor_tensor(s2r[:, :, 1, :], xr[:, :, 0, :], br[:, :, 1, :], mybir.AluOpType.mult)
                nc.vector.tensor_tensor(s1[:, 0:nh * K], s1[:, 0:nh * K], s2[:, 0:nh * K], mybir.AluOpType.add)
                nc.vector.tensor_copy(dst16, s1[:, 0:nh * K])

            rope(q16, qn, aq, bq, scr, scr2, R)
            rope(k16n, kn, ak, bk, scr, scr2, 1)

            # flatten only v's (b t) partition layout -> t-partitions (base 0)
            vfl = cpool.tile([T, BL * K], F16, tag="vfl")
            for t in range(T):
                nc.gpsimd.dma_start(vfl[t:t + 1, :], v16n[t::T, :])

            def vfl_s(b):
                return vfl[0:T, b * K:(b + 1) * K]

            # transpose q -> qT (128k x 16 qrow per b), k_new -> kTnew (128k x 4 per b)
            # qT via 4 whole-column transposes (64 tokens at once, base-0 aligned),
            # then one copy permuting (r b t) -> (b r t); kTnew via one transpose.
            qT = cpool.tile([128, BL * NQROW], F16, tag="qT")
            kTnew = cpool.tile([128, BL * T], F16, tag="kTnew")
            with tc.tile_pool(name="ps2", bufs=1, space="PSUM") as ps2:
                qTp = ps2.tile([128, BL * NQROW], F16, tag="qTp")
                kTnp = ps2.tile([128, BL * T], F16, tag="kTnp")
                for r in range(R):
                    nc.tensor.matmul(qTp[:, r * TOK:(r + 1) * TOK],
                                     q16[:, r * K:(r + 1) * K],
                                     ident16[0:TOK, 0:TOK], is_transpose=True,
                                     skip_group_check=True)
                nc.tensor.matmul(kTnp[:], k16n,
                                 ident16[0:TOK, 0:TOK], is_transpose=True,
                                 skip_group_check=True)
                nc.scalar.copy(
                    qT[:].rearrange("p (b r t) -> p b r t", b=BL, r=R),
                    qTp[:].rearrange("p (r b t) -> p b r t", r=R, b=BL))
                nc.scalar.copy(kTnew[:], kTnp[:])

            # ---------------- Fresh-token logits / exp / denom ---------------
            expfr = cpool.tile([T, BL * NQROW], F16, tag="expfr")
            freshden = cpool.tile([1, BL * NQROW], F32, tag="freshden")
            with tc.tile_pool(name="ps3", bufs=1, space="PSUM") as ps3:
                frp = ps3.tile([T, BL * NQROW], F32, tag="frp")
                fdp = ps3.tile([1, BL * NQROW], F32, tag="fdp")
                for b in range(BL):
                    nc.tensor.matmul(frp[0:T, b * NQROW:(b + 1) * NQROW],
                                     kTnew[:, b * T:(b + 1) * T],
                                     qT[:, b * NQROW:(b + 1) * NQROW],
                                     skip_group_check=True)
                nc.scalar.activation(expfr[:], frp[:], mybir.ActivationFunctionType.Exp,
                                     bias=c_neg1[0:T, 0:1], scale=float(SCALE))
                nc.vector.tensor_tensor(expfr[:], expfr[:], maskf[:], mybir.AluOpType.mult)
                nc.tensor.matmul(fdp[:], ones16[0:T, 0:1], expfr[:])
                nc.scalar.copy(freshden[:], fdp[:])

            # ---------------- Main attention loop over batches ----------------
            denall = cpool.tile([1, BL * NQROW], F32, tag="denall")
            dscr = cpool.tile([1, NQROW], F32, tag="dscr")
            attn_sb = cpool.tile([128, BL * NQROW], F16, tag="attn_sb")

            kvpool = ctx.enter_context(tc.tile_pool(name="kv", bufs=8))
            expool = ctx.enter_context(tc.tile_pool(name="expp", bufs=3))
            attn_ctx = ExitStack()
            lps = attn_ctx.enter_context(tc.tile_pool(name="lps", bufs=4, space="PSUM"))
            atps = attn_ctx.enter_context(tc.tile_pool(name="atps", bufs=1, space="PSUM"))
            dnps = attn_ctx.enter_context(tc.tile_pool(name="dnps", bufs=2, space="PSUM"))

            attn_ps = atps.tile([128, BL * NQROW], F32, tag="attnp")

            kv_tiles = []
            for b in range(BL):
                k16 = kvpool.tile([128, n_tiles * K], F8, tag="k16")
                v16 = kvpool.tile([128, n_tiles * K], F16, tag="v16")
                nc.sync.dma_start(k16[:], ext["kc"].ap()[b])
                nc.sync.dma_start(v16[:], ext["vc"].ap()[b])
                kv_tiles.append((k16, v16))

            for b in range(BL):
                k16, v16 = kv_tiles[b]
                lg = lps.tile([128, LCOLS], F32, tag="lg")
                # kc is host-packed transposed: k16[k, s] -- tile j is cols j*128..
                for j in range(n_tiles):
                    nc.tensor.matmul(lg[:, j * NQROW:(j + 1) * NQROW],
                                     k16[:, j * K:(j + 1) * K],
                                     qT[:, b * NQROW:(b + 1) * NQROW],
                                     skip_group_check=True)

                ex = expool.tile([128, LCOLS], F16, tag="ex")
                nc.scalar.activation(ex[:], lg[:], mybir.ActivationFunctionType.Exp,
                                     bias=c_neg1[:, 0:1], scale=float(SCALE))

                # denominator: ones-matmul partials + strided reduce + fresh part
                dn = dnps.tile([1, LCOLS], F32, tag="dn")
                nc.tensor.matmul(dn[:], ones16[:, 0:1], ex[:], skip_group_check=True)
                nc.vector.reduce_sum(dscr[:],
                                     dn[:].rearrange("p (j q) -> p q j", j=n_tiles),
                                     axis=mybir.AxisListType.X)
                nc.vector.tensor_tensor(denall[:, b * NQROW:(b + 1) * NQROW], dscr[:],
                                        freshden[:, b * NQROW:(b + 1) * NQROW],
                                        mybir.AluOpType.add)

                # A.V accumulation: fresh first, then cache tiles
                nc.tensor.matmul(attn_ps[:, b * NQROW:(b + 1) * NQROW],
                                 vfl_s(b),
                                 expfr[0:T, b * NQROW:(b + 1) * NQROW],
                                 start=True, stop=False, skip_group_check=True)
                for j in range(n_tiles):
                    nc.tensor.matmul(attn_ps[:, b * NQROW:(b + 1) * NQROW],
                                     v16[:, j * K:(j + 1) * K],
                                     ex[:, j * NQROW:(j + 1) * NQROW],
                                     start=False, stop=(j == n_tiles - 1),
                                     skip_group_check=True)

            # ---------------- Normalize + o_proj + output ---------------------
            recip = cpool.tile([1, BL * NQROW], F32, tag="recip")
            rbc = cpool.tile([128, BL * NQROW], F32, tag="rbc")
            nc.vector.reciprocal(recip[:], denall[:])
            bcps = attn_ctx.enter_context(tc.tile_pool(name="bcps", bufs=1, space="PSUM"))
            rbp = bcps.tile([128, BL * NQROW], F32, tag="rbp")
            nc.tensor.matmul(rbp[:], ones_row[:], recip[:])
            nc.scalar.copy(rbc[:], rbp[:])
            # normalize and permute (b r t) -> (r b t) so o_proj lhsT slices are contiguous
            nc.vector.tensor_tensor(
                attn_sb[:].rearrange("p (r b t) -> p b r t", r=R, b=BL),
                attn_ps[:].rearrange("p (b r t) -> p b r t", r=R, b=BL),
                rbc[:].rearrange("p (b r t) -> p b r t", r=R, b=BL),
                mybir.AluOpType.mult)
            attn_ctx.close()

            out_sb = cpool.tile([TOK, D], F32, tag="out_sb")
            with tc.tile_pool(name="ps4", bufs=1, space="PSUM") as ps4, \
                 tc.tile_pool(name="dram", bufs=1, space="DRAM") as dpool:
                outp = ps4.tile([TOK, D], F32, tag="outp")
                if use_collective:
                    bin_ = dpool.tile([TOK, D], F32, tag="bin")
                    bout = dpool.tile([TOK, D], F32, tag="bout")
                # pipelined per-bank: matmuls -> copy -> bounce DMA overlap
                for n in range(D // 512):
                    cs = slice(n * 512, (n + 1) * 512)
                    for r in range(R):
                        nc.tensor.matmul(outp[:, cs],
                                         attn_sb[:, r * TOK:(r + 1) * TOK],
                                         wo[:, r * D + n * 512: r * D + (n + 1) * 512],
                                         start=(r == 0), stop=(r == R - 1))
                    if n % 2 == 0:
                        nc.vector.tensor_copy(out_sb[:, cs], outp[:, cs])
                    else:
                        nc.scalar.copy(out_sb[:, cs], outp[:, cs])
                    if use_collective:
                        nc.sync.dma_start(bin_[:, cs], out_sb[:, cs])
                    else:
                        nc.sync.dma_start(out_ext.ap()[:, cs], out_sb[:, cs])
                if use_collective:
                    nc.gpsimd.collective_compute(
                        "AllReduce",
                        mybir.AluOpType.add,
                        replica_groups=[[0, 1, 2, 3], [4, 5, 6, 7]],
                        ins=[bin_.opt()],
                        outs=[bout.opt()],
                    )
                    nc.sync.dma_start(out_ext.ap(), bout[:])

    nc.compile()
    return nc


def _prepare_inputs(hidden_BTD, segment_ids_BT, k_cache, v_cache, Wq, Wk, Wv, Wo,
                    q_scale, k_scale, cur):
    """Host-side sharding/packing. Returns (in_maps, n_tiles)."""
    hidden = np.asarray(hidden_BTD, np.float32)
    seg = np.asarray(segment_ids_BT)
    kc = np.asarray(k_cache, np.float32)
    vc = np.asarray(v_cache, np.float32)
    Wq = np.asarray(Wq, np.float32)
    Wk = np.asarray(Wk, np.float32)
    Wv = np.asarray(Wv, np.float32)
    Wo = np.asarray(Wo, np.float32)
    q_scale = np.asarray(q_scale, np.float32)
    k_scale = np.asarray(k_scale, np.float32)

    assert cur % 128 == 0 and cur + T <= S, f"unsupported cur_ind {cur}"
    n_tiles = cur // 128

    # positions / pads, exactly as the reference
    valid = (seg != 0)
    csum = np.cumsum(valid.astype(np.int32), axis=-1)
    left_pads = np.sum((csum == 0).astype(np.int32), axis=-1)
    assert np.all(left_pads == 0) and np.all(seg == 1), "only dense segments supported"
    positions = (csum - 1).astype(np.float32) + np.float32(cur)    # (B,T)

    # rope sin/cos in fp32 as the reference computes them
    fraction = np.arange(0, K // 2, dtype=np.float32) * np.float32(2.0 / K)
    timescale = (np.float32(ROPE_BASE) ** fraction).astype(np.float32)
    sinusoid = positions[..., None] / timescale                     # (B,T,64)
    sin = np.sin(sinusoid).astype(np.float32)
    cos = np.cos(sinusoid).astype(np.float32)

    def rope_tables(scale_vec):
        # A[i]: coefficient of x[i]; B[i]: coefficient of x[swap(i)]
        A = np.concatenate([cos * scale_vec[:K // 2], cos * scale_vec[K // 2:]], axis=-1)
        Bc = np.concatenate([-sin * scale_vec[K // 2:], sin * scale_vec[:K // 2]], axis=-1)
        return A.astype(np.float32), Bc.astype(np.float32)          # (B,T,128)

    Aq, Bq = rope_tables(q_scale)
    Ak, Bk = rope_tables(k_scale)

    # fresh-token multiplicative causal mask: (t', b*16 + r*4 + t) -> t' <= t
    m = (np.arange(T)[:, None] <= np.arange(T)[None, :]).astype(np.float16)  # (t',t)
    maskf = np.tile(m[:, None, None, :], (1, BL, R, 1)).reshape(T, BL * R * T)

    in_maps = []
    for c in range(NCORES):
        bh, g = c // 4, c % 4
        bsl = slice(bh * BL, (bh + 1) * BL)
        hT = hidden[bsl].reshape(TOK, D).T                          # (2048, 64)
        qcols = slice(g * QCOLS, (g + 1) * QCOLS)
        kcols = slice(g * K, (g + 1) * K)
        kloc = kc[bsl, :cur, g, :]                                  # (16, cur, 128)
        vloc = vc[bsl, :cur, g, :]
        # K is packed TRANSPOSED (k on partitions): kpack[b, k, s]
        kpack = np.ascontiguousarray(kloc.transpose(0, 2, 1)).astype(F8NP)
        vpack = np.ascontiguousarray(
            vloc.reshape(BL, n_tiles, 128, K).transpose(0, 2, 1, 3).reshape(BL, 128, n_tiles * K)).astype(np.float16)
        aq_l = np.tile(Aq[bsl].reshape(TOK, K), (1, R))
        bq_l = np.tile(Bq[bsl].reshape(TOK, K), (1, R))
        in_maps.append({
            "hT": _pack_rows(np.ascontiguousarray(hT)).astype(np.float16),
            "wq": _pack_rows(np.ascontiguousarray(Wq[:, qcols] * WSCALE)).astype(F8NP),
            "wk": _pack_rows(np.ascontiguousarray(Wk[:, kcols] * WSCALE)).astype(F8NP),
            "wv": _pack_rows(np.ascontiguousarray(Wv[:, kcols] * WSCALE)).astype(F8NP),
            "wo": _pack_rows(np.ascontiguousarray(Wo[g * QCOLS:(g + 1) * QCOLS, :])).astype(np.float16),
            "kc": kpack,
            "vc": vpack,
            "aq": np.ascontiguousarray(aq_l),
            "bq": np.ascontiguousarray(bq_l),
            "ak": np.ascontiguousarray(Ak[bsl].reshape(TOK, K)),
            "bk": np.ascontiguousarray(Bk[bsl].reshape(TOK, K)),
            "ident16": np.eye(128, dtype=np.float16),
            "ones16": np.ones((128, 1), np.float16),
            "maskf": maskf,
        })
    return in_maps, n_tiles


def kernel(**inputs):
    cur = int(np.asarray(inputs["cur_ind"]))
    in_maps, n_tiles = _prepare_inputs(
        inputs["hidden_BTD"], inputs["segment_ids_BT"], inputs["k_cache"],
        inputs["v_cache"], inputs["Wq"], inputs["Wk"], inputs["Wv"], inputs["Wo"],
        inputs["q_scale"], inputs["k_scale"], cur)

    for use_coll in (True, False):
        key = (cur, use_coll, 1)
        try:
            if key not in _COMPILED:
                _COMPILED[key] = _build_nc(cur, n_tiles, use_coll, nreps=1)
            nc = _COMPILED[key]
            res = run_bass_kernel_spmd(nc, in_maps, list(range(NCORES)))
            outs = [res.results[c]["out"].reshape(BL, T, D) for c in range(NCORES)]
            if use_coll:
                full = np.concatenate([outs[0], outs[4]], axis=0)
            else:
                full = np.concatenate([sum(outs[0:4]), sum(outs[4:8])], axis=0)
            return full.astype(np.float32)
        except Exception:
            if not use_coll:
                raise
            import traceback
            traceback.print_exc()
            print("collective path failed; falling back to host-side reduce",
                  file=sys.stderr)
    raise RuntimeError("unreachable")



# revision 14
# speedup vs baseline: 657.4619x; 1.3034x over previous
"""Trainium2 Bass kernel for GQA decode attention (B=32,T=4,D=2048,H=16,G=4,K=128,S=4096).

Sharding: 8 NeuronCores = 2 batch-groups x 4 kv-head groups.
Core c: batches [16*(c//4), 16*(c//4)+16), kv head g = c % 4 (owns 4 q heads).
o_proj partial sums are AllReduce'd across each 4-core head group on device.

Device pipeline per core:
  - QKV projection (fp16 matmuls, fp32 PSUM accumulate)
  - RMS-norm + RoPE in fp32 on DVE/ACT (host-precomputed coefficient tables
    with q_scale/k_scale folded in)
  - attention in transposed orientation: PE-transpose K tiles, logits^T =
    kT @ qT (fp16 in, fp32 out), softmax WITHOUT max-subtraction -- safe
    because rms-normed q,k bound |logits| <= sqrt(K); exp computes
    exp(x/sqrt(K) - 1) on ACT (the -1 cancels in normalization and keeps
    fp16 exp values < 65504)
  - A.V accumulates attn^T directly (v natural layout as the stationary)
  - fp16 o_proj into fp32 PSUM, AllReduce, DMA out

Only cache rows [0, cur_ind) are read; rows [cur_ind, cur_ind+T) are the
freshly projected k/v handled on-chip, rows beyond are masked by the
reference -- so the cache update never materializes.
"""

import sys

sys.path.insert(0, "/opt/trn_rl_repo")

import numpy as np
import ml_dtypes

F8NP = ml_dtypes.float8_e3m4
WSCALE = 32.0

import concourse.bacc as bacc
import concourse.mybir as mybir
import concourse.tile as tile
from concourse.bass_utils import run_bass_kernel_spmd

F32 = mybir.dt.float32
F32R = mybir.dt.float32r
F16 = mybir.dt.float16
F8 = mybir.dt.float8e3   # TRN E3M4: ~1.6% normal / 2^-6 denormal step
# WSCALE (defined above): prescale for fp8 qkv weights; rms-norm cancels it
# for q/k, the fresh-v copy divides it back out

B, T, D = 32, 4, 2048
H, G, K = 16, 4, 128
S = 4096
R = H // G          # 4 q heads per kv head
EPS = 1e-6
ROPE_BASE = 10000.0
NCORES = 8
BG = 2              # batch groups
BL = B // BG        # 16 batches per core
TOK = BL * T        # 64 tokens per core
QCOLS = R * K       # 512 local q columns
NDC = D // 128      # 16 contraction chunks for qkv proj
SCALE = 1.0 / np.sqrt(np.float32(K))
EXP_BIAS = -1.0     # exp(x*SCALE + EXP_BIAS); cancels in softmax, avoids fp16 overflow

_COMPILED = {}


def _pack_rows(w):
    """(C*128, N) -> (128, C*N) with [p, c*N+n] = w[c*128+p, n]."""
    c = w.shape[0] // 128
    n = w.shape[1]
    return np.ascontiguousarray(
        w.reshape(c, 128, n).transpose(1, 0, 2).reshape(128, c * n)
    )


def _build_nc(cur, n_tiles, use_collective, nreps=1):
    """Build the kernel program. nreps>1 emits the complete body (all DRAM
    loads + compute + output) back-to-back that many times in one program —
    used by the benchmark harness to measure steady-state per-invocation HW
    time with dispatch overhead amortized. Every rep re-reads all inputs from
    DRAM and rewrites the output, so per-rep work is identical to nreps=1."""
    nc = bacc.Bacc("TRN2", target_bir_lowering=False, debug=False, num_devices=NCORES)

    ext = {}

    def inp(name, shape, dt=F32):
        ext[name] = nc.dram_tensor(name, list(shape), dt, kind="ExternalInput")
        return ext[name]

    inp("hT", (128, NDC * TOK), F16)       # hiddenT packed per d-chunk
    inp("wq", (128, NDC * QCOLS), F16)     # fp16: q noise lands 1:1 on the output
    inp("wk", (128, NDC * K), F8)          # k/v weights fp8 (affect fresh tokens only)
    inp("wv", (128, NDC * K), F8)
    inp("wo", (128, R * D), F16)
    inp("kc", (BL, 128, n_tiles * K), F8)  # cache K, host-packed (p = s%128), fp8
    inp("vc", (BL, 128, n_tiles * K), F16) # cache V stays fp16 (direct output error)
    inp("aq", (TOK, QCOLS))
    inp("bq", (TOK, QCOLS))
    inp("ak", (TOK, K))
    inp("bk", (TOK, K))
    inp("ident16", (128, 128), F16)
    inp("ones16", (128, 1), F16)
    inp("maskf", (T, BL * 4 * T), F16)     # multiplicative causal mask for fresh tokens
    out_ext = nc.dram_tensor("out", [TOK, D], F32, kind="ExternalOutput")

    NQROW = 4 * T                          # 16 query rows per batch (r*4+t)
    LCOLS = n_tiles * NQROW                # logitsT bank cols per batch

    with tile.TileContext(nc) as tc:
        from contextlib import ExitStack

        for _rep in range(nreps):
          with ExitStack() as ctx:
            cpool = ctx.enter_context(tc.tile_pool(name="const", bufs=1))

            def load(name, dt=None, eng=None, split=1):
                h = ext[name]
                t_ = cpool.tile(list(h.shape), dt or h.dtype, tag=name)
                ncols = h.shape[-1]
                step = ncols // split
                for s0 in range(0, ncols, step):
                    (eng or nc.sync).dma_start(t_[:, s0:s0 + step], h.ap()[:, s0:s0 + step])
                return t_

            hT = load("hT", split=4)
            wq = load("wq", split=4)
            wk = load("wk", split=4)
            wv = load("wv", split=4)
            wo = load("wo", split=2)
            aq = load("aq")
            bq = load("bq")
            ak = load("ak")
            bk = load("bk")
            ident16 = load("ident16")
            ones16 = load("ones16")
            maskf = load("maskf")

            # ---------------- Phase 1: QKV projection + norm + rope ----------
            c_eps = cpool.tile([128, 1], F32, tag="c_eps")
            c_neg1 = cpool.tile([128, 1], F32, tag="c_neg1")
            nc.vector.memset(c_eps[:], float(EPS))
            nc.vector.memset(c_neg1[:], float(EXP_BIAS))
            ones_row = cpool.tile([1, 128], F32, tag="ones_row")
            nc.vector.memset(ones_row[:], 1.0)

            qn = cpool.tile([TOK, QCOLS], F32, tag="qn")       # normed+roped q
            kn = cpool.tile([TOK, K], F32, tag="kn")
            WQKV = QCOLS + 2 * K                               # 768 combined cols
            qkv16 = cpool.tile([TOK, WQKV], F16, tag="qkv16")
            q16 = qkv16[:, 0:QCOLS]
            k16n = qkv16[:, QCOLS:QCOLS + K]
            v16n = qkv16[:, QCOLS + K:WQKV]
            ssq = cpool.tile([TOK, 8], F32, tag="ssq")
            rstd = cpool.tile([TOK, 8], F32, tag="rstd")
            scr = cpool.tile([TOK, QCOLS], F32, tag="scr")
            scr2 = cpool.tile([TOK, QCOLS], F32, tag="scr2")

            with tc.tile_pool(name="ps1", bufs=1, space="PSUM") as ps1:
                pq = ps1.tile([TOK, QCOLS], F32, tag="pq")
                pk = ps1.tile([TOK, K], F32, tag="pk")
                pv = ps1.tile([TOK, K], F32, tag="pv")
                for c in range(NDC):
                    lhs = hT[:, c * TOK:(c + 1) * TOK]
                    st, sp = (c == 0), (c == NDC - 1)
                    nc.tensor.matmul(pq[:], lhs, wq[:, c * QCOLS:(c + 1) * QCOLS], start=st, stop=sp)
                    nc.tensor.matmul(pk[:], lhs, wk[:, c * K:(c + 1) * K], start=st, stop=sp)
                    nc.tensor.matmul(pv[:], lhs, wv[:, c * K:(c + 1) * K], start=st, stop=sp)

                # sum of squares per (token, head)
                for h in range(R):
                    nc.scalar.activation(scr[:, h * K:(h + 1) * K], pq[:, h * K:(h + 1) * K],
                                         mybir.ActivationFunctionType.Square,
                                         accum_out=ssq[:, h:h + 1])
                nc.scalar.activation(scr2[:, 0:K], pk[:],
                                     mybir.ActivationFunctionType.Square,
                                     accum_out=ssq[:, R:R + 1])
                # std = sqrt(ssq/K + eps); rstd = 1/std
                nc.scalar.activation(rstd[:, 0:5], ssq[:, 0:5],
                                     mybir.ActivationFunctionType.Sqrt,
                                     bias=c_eps[0:TOK, 0:1], scale=float(1.0 / K))
                nc.vector.reciprocal(ssq[:, 0:5], rstd[:, 0:5])
                # q_hat = q * rstd (per token/head), same for k
                for h in range(R):
                    nc.vector.tensor_scalar(qn[:, h * K:(h + 1) * K], pq[:, h * K:(h + 1) * K],
                                            ssq[:, h:h + 1], None, mybir.AluOpType.mult)
                nc.vector.tensor_scalar(kn[:], pk[:], ssq[:, R:R + 1], None, mybir.AluOpType.mult)
                # v -> fp16, undoing the fp8 weight prescale
                nc.scalar.activation(v16n, pv[:],
                                     mybir.ActivationFunctionType.Copy,
                                     scale=float(1.0 / WSCALE))

            # rope: out = qh*A + swap_halves(qh)*B   (scale folded into A/B)
            def rope(dst16, x, a, b, s1, s2, nh):
                xr = x[:].rearrange("p (h u x) -> p h u x", h=nh, u=2)
                # dst16 is an AP slice of the combined qkv16 tile
                br = b[:].rearrange("p (h u x) -> p h u x", h=nh, u=2)
                s2r = s2[:, 0:nh * K].rearrange("p (h u x) -> p h u x", h=nh, u=2)
                nc.vector.tensor_tensor(s1[:, 0:nh * K], x[:, 0:nh * K], a[:, 0:nh * K], mybir.AluOpType.mult)
                # swapped-half products
                nc.vector.tensor_tensor(s2r[:, :, 0, :], xr[:, :, 1, :], br[:, :, 0, :], mybir.AluOpType.mult)
                nc.vector.tensor_tensor(s2r[:, :, 1, :], xr[:, :, 0, :], br[:, :, 1, :], mybir.AluOpType.mult)
                nc.vector.tensor_tensor(s1[:, 0:nh * K], s1[:, 0:nh * K], s2[:, 0:nh * K], mybir.AluOpType.add)
                nc.vector.tensor_copy(dst16, s1[:, 0:nh * K])

            rope(q16, qn, aq, bq, scr, scr2, R)
            rope(k16n, kn, ak, bk, scr, scr2, 1)

            # flatten only v's (b t) partition layout -> t-partitions (base 0)
            vfl = cpool.tile([T, BL * K], F16, tag="vfl")
            for t in range(T):
                nc.gpsimd.dma_start(vfl[t:t + 1, :], v16n[t::T, :])

            def vfl_s(b):
                return vfl[0:T, b * K:(b + 1) * K]

            # transpose q -> qT (128k x 16 qrow per b), k_new -> kTnew (128k x 4 per b)
            # qT via 4 whole-column transposes (64 tokens at once, base-0 aligned),
            # then one copy permuting (r b t) -> (b r t); kTnew via one transpose.
            qT = cpool.tile([128, BL * NQROW], F16, tag="qT")
            kTnew = cpool.tile([128, BL * T], F16, tag="kTnew")
            with tc.tile_pool(name="ps2", bufs=1, space="PSUM") as ps2:
                qTp = ps2.tile([128, BL * NQROW], F16, tag="qTp")
                kTnp = ps2.tile([128, BL * T], F16, tag="kTnp")
                for r in range(R):
                    nc.tensor.matmul(qTp[:, r * TOK:(r + 1) * TOK],
                                     q16[:, r * K:(r + 1) * K],
                                     ident16[0:TOK, 0:TOK], is_transpose=True,
                                     skip_group_check=True)
                nc.tensor.matmul(kTnp[:], k16n,
                                 ident16[0:TOK, 0:TOK], is_transpose=True,
                                 skip_group_check=True)
                nc.scalar.copy(
                    qT[:].rearrange("p (b r t) -> p b r t", b=BL, r=R),
                    qTp[:].rearrange("p (r b t) -> p b r t", r=R, b=BL))
                nc.scalar.copy(kTnew[:], kTnp[:])

            # ---------------- Fresh-token logits / exp / denom ---------------
            expfr = cpool.tile([T, BL * NQROW], F16, tag="expfr")
            freshden = cpool.tile([1, BL * NQROW], F32, tag="freshden")
            with tc.tile_pool(name="ps3", bufs=1, space="PSUM") as ps3:
                frp = ps3.tile([T, BL * NQROW], F32, tag="frp")
                fdp = ps3.tile([1, BL * NQROW], F32, tag="fdp")
                for b in range(BL):
                    nc.tensor.matmul(frp[0:T, b * NQROW:(b + 1) * NQROW],
                                     kTnew[:, b * T:(b + 1) * T],
                                     qT[:, b * NQROW:(b + 1) * NQROW],
                                     skip_group_check=True)
                nc.scalar.activation(expfr[:], frp[:], mybir.ActivationFunctionType.Exp,
                                     bias=c_neg1[0:T, 0:1], scale=float(SCALE))
                nc.vector.tensor_tensor(expfr[:], expfr[:], maskf[:], mybir.AluOpType.mult)
                nc.tensor.matmul(fdp[:], ones16[0:T, 0:1], expfr[:])
                nc.scalar.copy(freshden[:], fdp[:])

            # ---------------- Main attention loop over batches ----------------
            denall = cpool.tile([1, BL * NQROW], F32, tag="denall")
            dscr = cpool.tile([1, NQROW], F32, tag="dscr")
            attn_sb = cpool.tile([128, BL * NQROW], F16, tag="attn_sb")

            kvpool = ctx.enter_context(tc.tile_pool(name="kv", bufs=8))
            expool = ctx.enter_context(tc.tile_pool(name="expp", bufs=3))
            attn_ctx = ExitStack()
            lps = attn_ctx.enter_context(tc.tile_pool(name="lps", bufs=4, space="PSUM"))
            atps = attn_ctx.enter_context(tc.tile_pool(name="atps", bufs=1, space="PSUM"))
            dnps = attn_ctx.enter_context(tc.tile_pool(name="dnps", bufs=2, space="PSUM"))

            attn_ps = atps.tile([128, BL * NQROW], F32, tag="attnp")

            kv_tiles = []
            for b in range(BL):
                k16 = kvpool.tile([128, n_tiles * K], F8, tag="k16")
                v16 = kvpool.tile([128, n_tiles * K], F16, tag="v16")
                nc.sync.dma_start(k16[:], ext["kc"].ap()[b])
                nc.sync.dma_start(v16[:], ext["vc"].ap()[b])
                kv_tiles.append((k16, v16))

            for b in range(BL):
                k16, v16 = kv_tiles[b]
                lg = lps.tile([128, LCOLS], F32, tag="lg")
                # kc is host-packed transposed: k16[k, s] -- tile j is cols j*128..
                for j in range(n_tiles):
                    nc.tensor.matmul(lg[:, j * NQROW:(j + 1) * NQROW],
                                     k16[:, j * K:(j + 1) * K],
                                     qT[:, b * NQROW:(b + 1) * NQROW],
                                     skip_group_check=True)

                ex = expool.tile([128, LCOLS], F16, tag="ex")
                nc.scalar.activation(ex[:], lg[:], mybir.ActivationFunctionType.Exp,
                                     bias=c_neg1[:, 0:1], scale=float(SCALE))

                # denominator: ones-matmul partials + strided reduce + fresh part
                dn = dnps.tile([1, LCOLS], F32, tag="dn")
                nc.tensor.matmul(dn[:], ones16[:, 0:1], ex[:], skip_group_check=True)
                nc.vector.reduce_sum(dscr[:],
                                     dn[:].rearrange("p (j q) -> p q j", j=n_tiles),
                                     axis=mybir.AxisListType.X)
                nc.vector.tensor_tensor(denall[:, b * NQROW:(b + 1) * NQROW], dscr[:],
                                        freshden[:, b * NQROW:(b + 1) * NQROW],
                                        mybir.AluOpType.add)

                # A.V accumulation: fresh first, then cache tiles
                nc.tensor.matmul(attn_ps[:, b * NQROW:(b + 1) * NQROW],
                                 vfl_s(b),
                                 expfr[0:T, b * NQROW:(b + 1) * NQROW],
                                 start=True, stop=False, skip_group_check=True)
                for j in range(n_tiles):
                    nc.tensor.matmul(attn_ps[:, b * NQROW:(b + 1) * NQROW],
                                     v16[:, j * K:(j + 1) * K],
                                     ex[:, j * NQROW:(j + 1) * NQROW],
                                     start=False, stop=(j == n_tiles - 1),
                                     skip_group_check=True)

            # ---------------- Normalize + o_proj + output ---------------------
            recip = cpool.tile([1, BL * NQROW], F32, tag="recip")
            rbc = cpool.tile([128, BL * NQROW], F32, tag="rbc")
            nc.vector.reciprocal(recip[:], denall[:])
            bcps = attn_ctx.enter_context(tc.tile_pool(name="bcps", bufs=1, space="PSUM"))
            rbp = bcps.tile([128, BL * NQROW], F32, tag="rbp")
            nc.tensor.matmul(rbp[:], ones_row[:], recip[:])
            nc.scalar.copy(rbc[:], rbp[:])
            # normalize and permute (b r t) -> (r b t) so o_proj lhsT slices are contiguous
            nc.vector.tensor_tensor(
                attn_sb[:].rearrange("p (r b t) -> p b r t", r=R, b=BL),
                attn_ps[:].rearrange("p (b r t) -> p b r t", r=R, b=BL),
                rbc[:].rearrange("p (b r t) -> p b r t", r=R, b=BL),
                mybir.AluOpType.mult)
            attn_ctx.close()

            out_sb = cpool.tile([TOK, D], F32, tag="out_sb")
            with tc.tile_pool(name="ps4", bufs=1, space="PSUM") as ps4, \
                 tc.tile_pool(name="dram", bufs=1, space="DRAM") as dpool:
                outp = ps4.tile([TOK, D], F32, tag="outp")
                if use_collective:
                    bin_ = dpool.tile([TOK, D], F32, tag="bin")
                    bout = dpool.tile([TOK, D], F32, tag="bout")
                # pipelined per-bank: matmuls -> copy -> bounce DMA overlap
                for n in range(D // 512):
                    cs = slice(n * 512, (n + 1) * 512)
                    for r in range(R):
                        nc.tensor.matmul(outp[:, cs],
                                         attn_sb[:, r * TOK:(r + 1) * TOK],
                                         wo[:, r * D + n * 512: r * D + (n + 1) * 512],
                                         start=(r == 0), stop=(r == R - 1))
                    if n % 2 == 0:
                        nc.vector.tensor_copy(out_sb[:, cs], outp[:, cs])
                    else:
                        nc.scalar.copy(out_sb[:, cs], outp[:, cs])
                    if use_collective:
                        nc.sync.dma_start(bin_[:, cs], out_sb[:, cs])
                    else:
                        nc.sync.dma_start(out_ext.ap()[:, cs], out_sb[:, cs])
                if use_collective:
                    nc.gpsimd.collective_compute(
                        "AllReduce",
                        mybir.AluOpType.add,
                        replica_groups=[[0, 1, 2, 3], [4, 5, 6, 7]],
                        ins=[bin_.opt()],
                        outs=[bout.opt()],
                    )
                    nc.sync.dma_start(out_ext.ap(), bout[:])

    nc.compile()
    return nc


def _prepare_inputs(hidden_BTD, segment_ids_BT, k_cache, v_cache, Wq, Wk, Wv, Wo,
                    q_scale, k_scale, cur):
    """Host-side sharding/packing. Returns (in_maps, n_tiles)."""
    hidden = np.asarray(hidden_BTD, np.float32)
    seg = np.asarray(segment_ids_BT)
    kc = np.asarray(k_cache, np.float32)
    vc = np.asarray(v_cache, np.float32)
    Wq = np.asarray(Wq, np.float32)
    Wk = np.asarray(Wk, np.float32)
    Wv = np.asarray(Wv, np.float32)
    Wo = np.asarray(Wo, np.float32)
    q_scale = np.asarray(q_scale, np.float32)
    k_scale = np.asarray(k_scale, np.float32)

    assert cur % 128 == 0 and cur + T <= S, f"unsupported cur_ind {cur}"
    n_tiles = cur // 128

    # positions / pads, exactly as the reference
    valid = (seg != 0)
    csum = np.cumsum(valid.astype(np.int32), axis=-1)
    left_pads = np.sum((csum == 0).astype(np.int32), axis=-1)
    assert np.all(left_pads == 0) and np.all(seg == 1), "only dense segments supported"
    positions = (csum - 1).astype(np.float32) + np.float32(cur)    # (B,T)

    # rope sin/cos in fp32 as the reference computes them
    fraction = np.arange(0, K // 2, dtype=np.float32) * np.float32(2.0 / K)
    timescale = (np.float32(ROPE_BASE) ** fraction).astype(np.float32)
    sinusoid = positions[..., None] / timescale                     # (B,T,64)
    sin = np.sin(sinusoid).astype(np.float32)
    cos = np.cos(sinusoid).astype(np.float32)

    def rope_tables(scale_vec):
        # A[i]: coefficient of x[i]; B[i]: coefficient of x[swap(i)]
        A = np.concatenate([cos * scale_vec[:K // 2], cos * scale_vec[K // 2:]], axis=-1)
        Bc = np.concatenate([-sin * scale_vec[K // 2:], sin * scale_vec[:K // 2]], axis=-1)
        return A.astype(np.float32), Bc.astype(np.float32)          # (B,T,128)

    Aq, Bq = rope_tables(q_scale)
    Ak, Bk = rope_tables(k_scale)

    # fresh-token multiplicative causal mask: (t', b*16 + r*4 + t) -> t' <= t
    m = (np.arange(T)[:, None] <= np.arange(T)[None, :]).astype(np.float16)  # (t',t)
    maskf = np.tile(m[:, None, None, :], (1, BL, R, 1)).reshape(T, BL * R * T)

    in_maps = []
    for c in range(NCORES):
        bh, g = c // 4, c % 4
        bsl = slice(bh * BL, (bh + 1) * BL)
        hT = hidden[bsl].reshape(TOK, D).T                          # (2048, 64)
        qcols = slice(g * QCOLS, (g + 1) * QCOLS)
        kcols = slice(g * K, (g + 1) * K)
        kloc = kc[bsl, :cur, g, :]                                  # (16, cur, 128)
        vloc = vc[bsl, :cur, g, :]
        # K is packed TRANSPOSED (k on partitions): kpack[b, k, s]
        kpack = np.ascontiguousarray(kloc.transpose(0, 2, 1)).astype(F8NP)
        vpack = np.ascontiguousarray(
            vloc.reshape(BL, n_tiles, 128, K).transpose(0, 2, 1, 3).reshape(BL, 128, n_tiles * K)).astype(np.float16)
        aq_l = np.tile(Aq[bsl].reshape(TOK, K), (1, R))
        bq_l = np.tile(Bq[bsl].reshape(TOK, K), (1, R))
        in_maps.append({
            "hT": _pack_rows(np.ascontiguousarray(hT)).astype(np.float16),
            "wq": _pack_rows(np.ascontiguousarray(Wq[:, qcols])).astype(np.float16),
            "wk": _pack_rows(np.ascontiguousarray(Wk[:, kcols] * WSCALE)).astype(F8NP),
            "wv": _pack_rows(np.ascontiguousarray(Wv[:, kcols] * WSCALE)).astype(F8NP),
            "wo": _pack_rows(np.ascontiguousarray(Wo[g * QCOLS:(g + 1) * QCOLS, :])).astype(np.float16),
            "kc": kpack,
            "vc": vpack,
            "aq": np.ascontiguousarray(aq_l),
            "bq": np.ascontiguousarray(bq_l),
            "ak": np.ascontiguousarray(Ak[bsl].reshape(TOK, K)),
            "bk": np.ascontiguousarray(Bk[bsl].reshape(TOK, K)),
            "ident16": np.eye(128, dtype=np.float16),
            "ones16": np.ones((128, 1), np.float16),
            "maskf": maskf,
        })
    return in_maps, n_tiles


def kernel(**inputs):
    cur = int(np.asarray(inputs["cur_ind"]))
    in_maps, n_tiles = _prepare_inputs(
        inputs["hidden_BTD"], inputs["segment_ids_BT"], inputs["k_cache"],
        inputs["v_cache"], inputs["Wq"], inputs["Wk"], inputs["Wv"], inputs["Wo"],
        inputs["q_scale"], inputs["k_scale"], cur)

    for use_coll in (True, False):
        key = (cur, use_coll, 1)
        try:
            if key not in _COMPILED:
                _COMPILED[key] = _build_nc(cur, n_tiles, use_coll, nreps=1)
            nc = _COMPILED[key]
            res = run_bass_kernel_spmd(nc, in_maps, list(range(NCORES)))
            outs = [res.results[c]["out"].reshape(BL, T, D) for c in range(NCORES)]
            if use_coll:
                full = np.concatenate([outs[0], outs[4]], axis=0)
            else:
                full = np.concatenate([sum(outs[0:4]), sum(outs[4:8])], axis=0)
            return full.astype(np.float32)
        except Exception:
            if not use_coll:
                raise
            import traceback
            traceback.print_exc()
            print("collective path failed; falling back to host-side reduce",
                  file=sys.stderr)
    raise RuntimeError("unreachable")



# revision 21
# speedup vs baseline: 746.2880x; 1.1351x over previous
"""Trainium2 Bass kernel for GQA decode attention (B=32,T=4,D=2048,H=16,G=4,K=128,S=4096).

Sharding: 8 NeuronCores = 2 batch-groups x 4 kv-head groups.
Core c: batches [16*(c//4), 16*(c//4)+16), kv head g = c % 4 (owns 4 q heads).
o_proj partial sums are AllReduce'd across each 4-core head group on device.

Device pipeline per core:
  - QKV projection (fp16 matmuls, fp32 PSUM accumulate)
  - RMS-norm + RoPE in fp32 on DVE/ACT (host-precomputed coefficient tables
    with q_scale/k_scale folded in)
  - attention in transposed orientation: PE-transpose K tiles, logits^T =
    kT @ qT (fp16 in, fp32 out), softmax WITHOUT max-subtraction -- safe
    because rms-normed q,k bound |logits| <= sqrt(K); exp computes
    exp(x/sqrt(K) - 1) on ACT (the -1 cancels in normalization and keeps
    fp16 exp values < 65504)
  - A.V accumulates attn^T directly (v natural layout as the stationary)
  - fp16 o_proj into fp32 PSUM, AllReduce, DMA out

Only cache rows [0, cur_ind) are read; rows [cur_ind, cur_ind+T) are the
freshly projected k/v handled on-chip, rows beyond are masked by the
reference -- so the cache update never materializes.
"""

import sys

sys.path.insert(0, "/opt/trn_rl_repo")

import numpy as np
import ml_dtypes

F8NP = ml_dtypes.float8_e3m4
WSCALE = 32.0

import concourse.bacc as bacc
import concourse.mybir as mybir
import concourse.tile as tile
from concourse.bass_utils import run_bass_kernel_spmd

F32 = mybir.dt.float32
F32R = mybir.dt.float32r
F16 = mybir.dt.float16
F8 = mybir.dt.float8e3   # TRN E3M4: ~1.6% normal / 2^-6 denormal step
# WSCALE (defined above): prescale for fp8 qkv weights; rms-norm cancels it
# for q/k, the fresh-v copy divides it back out

B, T, D = 32, 4, 2048
H, G, K = 16, 4, 128
S = 4096
R = H // G          # 4 q heads per kv head
EPS = 1e-6
ROPE_BASE = 10000.0
NCORES = 8
BG = 2              # batch groups
BL = B // BG        # 16 batches per core
TOK = BL * T        # 64 tokens per core
QCOLS = R * K       # 512 local q columns
NDC = D // 128      # 16 contraction chunks for qkv proj
SCALE = 1.0 / np.sqrt(np.float32(K))
EXP_BIAS = -1.0     # exp(x*SCALE + EXP_BIAS); cancels in softmax, avoids fp16 overflow

_COMPILED = {}


def _pack_rows(w):
    """(C*128, N) -> (128, C*N) with [p, c*N+n] = w[c*128+p, n]."""
    c = w.shape[0] // 128
    n = w.shape[1]
    return np.ascontiguousarray(
        w.reshape(c, 128, n).transpose(1, 0, 2).reshape(128, c * n)
    )


def _build_nc(cur, n_tiles, use_collective, nreps=1):
    """Build the kernel program. nreps>1 emits the complete body (all DRAM
    loads + compute + output) back-to-back that many times in one program —
    used by the benchmark harness to measure steady-state per-invocation HW
    time with dispatch overhead amortized. Every rep re-reads all inputs from
    DRAM and rewrites the output, so per-rep work is identical to nreps=1."""
    nc = bacc.Bacc("TRN2", target_bir_lowering=False, debug=False, num_devices=NCORES)

    ext = {}

    def inp(name, shape, dt=F32):
        ext[name] = nc.dram_tensor(name, list(shape), dt, kind="ExternalInput")
        return ext[name]

    inp("hT", (128, NDC * TOK), F16)       # hiddenT packed per d-chunk
    inp("wq", (128, NDC * QCOLS), F16)     # fp16: q noise lands 1:1 on the output
    inp("wkv", (128, NDC * 2 * K), F8)     # [wk|wv] fp8 (affect fresh tokens only)
    inp("wo", (128, R * D), F16)
    inp("kc", (BL, 128, n_tiles * K), F8)  # cache, host-packed (p = s%128), fp8
    inp("vc", (BL, 128, n_tiles * K), F8)  # e3m4: ~1.3% rms; FWL keeps A.V at ~76ns/mm
    inp("aq", (TOK, QCOLS))
    inp("bq", (TOK, QCOLS))
    inp("ak", (TOK, K))
    inp("bk", (TOK, K))
    inp("ident16", (128, 128), F16)
    inp("ones16", (128, 1), F16)
    inp("maskf", (T, BL * 4 * T), F16)     # multiplicative causal mask for fresh tokens
    out_ext = nc.dram_tensor("out", [TOK, D], F32, kind="ExternalOutput")

    NQROW = 4 * T                          # 16 query rows per batch (r*4+t)
    LCOLS = n_tiles * NQROW                # logitsT bank cols per batch

    with tile.TileContext(nc) as tc:
        from contextlib import ExitStack

        for _rep in range(nreps):
          with ExitStack() as ctx:
            cpool = ctx.enter_context(tc.tile_pool(name="const", bufs=1))

            def load(name, dt=None, eng=None, split=1):
                h = ext[name]
                t_ = cpool.tile(list(h.shape), dt or h.dtype, tag=name)
                ncols = h.shape[-1]
                step = ncols // split
                for s0 in range(0, ncols, step):
                    (eng or nc.sync).dma_start(t_[:, s0:s0 + step], h.ap()[:, s0:s0 + step])
                return t_

            hT = load("hT", split=4)
            wq = load("wq", split=4)
            wkv = load("wkv", split=4)
            wo = load("wo", split=2)
            aq = load("aq")
            bq = load("bq")
            ak = load("ak")
            bk = load("bk")
            ident16 = load("ident16")
            ones16 = load("ones16")
            maskf = load("maskf")

            # ---------------- Phase 1: QKV projection + norm + rope ----------
            c_eps = cpool.tile([128, 1], F32, tag="c_eps")
            c_neg1 = cpool.tile([128, 1], F32, tag="c_neg1")
            nc.vector.memset(c_eps[:], float(EPS))
            nc.vector.memset(c_neg1[:], float(EXP_BIAS))
            ones_row = cpool.tile([1, 128], F32, tag="ones_row")
            nc.vector.memset(ones_row[:], 1.0)

            qn = cpool.tile([TOK, QCOLS], F32, tag="qn")       # normed+roped q
            kn = cpool.tile([TOK, K], F32, tag="kn")
            WQKV = QCOLS + 2 * K                               # 768 combined cols
            qkv16 = cpool.tile([TOK, WQKV], F16, tag="qkv16")
            q16 = qkv16[:, 0:QCOLS]
            k16n = qkv16[:, QCOLS:QCOLS + K]
            v16n = qkv16[:, QCOLS + K:WQKV]
            ssq = cpool.tile([TOK, 8], F32, tag="ssq")
            rstd = cpool.tile([TOK, 8], F32, tag="rstd")
            scr = cpool.tile([TOK, QCOLS], F32, tag="scr")
            scr2 = cpool.tile([TOK, QCOLS], F32, tag="scr2")

            with tc.tile_pool(name="ps1", bufs=1, space="PSUM") as ps1:
                pq = ps1.tile([TOK, QCOLS], F32, tag="pq")
                pkv = ps1.tile([TOK, 2 * K], F32, tag="pkv")
                pk = pkv[:, 0:K]
                pv = pkv[:, K:2 * K]
                for c in range(NDC):
                    lhs = hT[:, c * TOK:(c + 1) * TOK]
                    st, sp = (c == 0), (c == NDC - 1)
                    nc.tensor.matmul(pq[:], lhs, wq[:, c * QCOLS:(c + 1) * QCOLS], start=st, stop=sp)
                    nc.tensor.matmul(pkv[:], lhs, wkv[:, c * 2 * K:(c + 1) * 2 * K], start=st, stop=sp)

                # sum of squares per (token, head)
                for h in range(R):
                    nc.scalar.activation(scr[:, h * K:(h + 1) * K], pq[:, h * K:(h + 1) * K],
                                         mybir.ActivationFunctionType.Square,
                                         accum_out=ssq[:, h:h + 1])
                nc.scalar.activation(scr2[:, 0:K], pk,
                                     mybir.ActivationFunctionType.Square,
                                     accum_out=ssq[:, R:R + 1])
                # std = sqrt(ssq/K + eps); rstd = 1/std
                nc.scalar.activation(rstd[:, 0:5], ssq[:, 0:5],
                                     mybir.ActivationFunctionType.Sqrt,
                                     bias=c_eps[0:TOK, 0:1], scale=float(1.0 / K))
                nc.vector.reciprocal(ssq[:, 0:5], rstd[:, 0:5])
                # q_hat = q * rstd (per token/head), same for k
                for h in range(R):
                    nc.vector.tensor_scalar(qn[:, h * K:(h + 1) * K], pq[:, h * K:(h + 1) * K],
                                            ssq[:, h:h + 1], None, mybir.AluOpType.mult)
                nc.vector.tensor_scalar(kn[:], pk, ssq[:, R:R + 1], None, mybir.AluOpType.mult)
                # v -> fp16, undoing the fp8 weight prescale
                nc.scalar.activation(v16n, pv,
                                     mybir.ActivationFunctionType.Copy,
                                     scale=float(1.0 / WSCALE))

            # rope: out = qh*A + swap_halves(qh)*B   (scale folded into A/B)
            def rope(dst16, x, a, b, s1, s2, nh):
                xr = x[:].rearrange("p (h u x) -> p h u x", h=nh, u=2)
                # dst16 is an AP slice of the combined qkv16 tile
                br = b[:].rearrange("p (h u x) -> p h u x", h=nh, u=2)
                s2r = s2[:, 0:nh * K].rearrange("p (h u x) -> p h u x", h=nh, u=2)
                nc.vector.tensor_tensor(s1[:, 0:nh * K], x[:, 0:nh * K], a[:, 0:nh * K], mybir.AluOpType.mult)
                # swapped-half products
                nc.vector.tensor_tensor(s2r[:, :, 0, :], xr[:, :, 1, :], br[:, :, 0, :], mybir.AluOpType.mult)
                nc.vector.tensor_tensor(s2r[:, :, 1, :], xr[:, :, 0, :], br[:, :, 1, :], mybir.AluOpType.mult)
                nc.vector.tensor_tensor(s1[:, 0:nh * K], s1[:, 0:nh * K], s2[:, 0:nh * K], mybir.AluOpType.add)
                nc.vector.tensor_copy(dst16, s1[:, 0:nh * K])

            rope(q16, qn, aq, bq, scr, scr2, R)
            rope(k16n, kn, ak, bk, scr, scr2, 1)

            # flatten only v's (b t) partition layout -> t-partitions (base 0)
            vfl = cpool.tile([T, BL * K], F16, tag="vfl")
            for t in range(T):
                nc.gpsimd.dma_start(vfl[t:t + 1, :], v16n[t::T, :])

            def vfl_s(b):
                return vfl[0:T, b * K:(b + 1) * K]

            # transpose q -> qT (128k x 16 qrow per b), k_new -> kTnew (128k x 4 per b)
            # qT via 4 whole-column transposes (64 tokens at once, base-0 aligned),
            # then one copy permuting (r b t) -> (b r t); kTnew via one transpose.
            qT = cpool.tile([128, BL * NQROW], F16, tag="qT")
            kTnew = cpool.tile([128, BL * T], F16, tag="kTnew")
            with tc.tile_pool(name="ps2", bufs=1, space="PSUM") as ps2:
                qTp = ps2.tile([128, BL * NQROW], F16, tag="qTp")
                kTnp = ps2.tile([128, BL * T], F16, tag="kTnp")
                for r in range(R):
                    nc.tensor.matmul(qTp[:, r * TOK:(r + 1) * TOK],
                                     q16[:, r * K:(r + 1) * K],
                                     ident16[0:TOK, 0:TOK], is_transpose=True,
                                     skip_group_check=True)
                nc.tensor.matmul(kTnp[:], k16n,
                                 ident16[0:TOK, 0:TOK], is_transpose=True,
                                 skip_group_check=True)
                nc.scalar.copy(
                    qT[:].rearrange("p (b r t) -> p b r t", b=BL, r=R),
                    qTp[:].rearrange("p (r b t) -> p b r t", r=R, b=BL))
                nc.scalar.copy(kTnew[:], kTnp[:])

            # ---------------- Fresh-token logits / exp / denom ---------------
            expfr = cpool.tile([T, BL * NQROW], F16, tag="expfr")
            freshden = cpool.tile([1, BL * NQROW], F32, tag="freshden")
            with tc.tile_pool(name="ps3", bufs=1, space="PSUM") as ps3:
                frp = ps3.tile([T, BL * NQROW], F32, tag="frp")
                fdp = ps3.tile([1, BL * NQROW], F32, tag="fdp")
                for b in range(BL):
                    nc.tensor.matmul(frp[0:T, b * NQROW:(b + 1) * NQROW],
                                     kTnew[:, b * T:(b + 1) * T],
                                     qT[:, b * NQROW:(b + 1) * NQROW],
                                     skip_group_check=True)
                nc.scalar.activation(expfr[:], frp[:], mybir.ActivationFunctionType.Exp,
                                     bias=c_neg1[0:T, 0:1], scale=float(SCALE))
                nc.vector.tensor_tensor(expfr[:], expfr[:], maskf[:], mybir.AluOpType.mult)
                nc.tensor.matmul(fdp[:], ones16[0:T, 0:1], expfr[:])
                nc.scalar.copy(freshden[:], fdp[:])

            # ---------------- Main attention loop over batches ----------------
            denall = cpool.tile([1, BL * NQROW], F32, tag="denall")
            dscr = cpool.tile([1, NQROW], F32, tag="dscr")
            attn_sb = cpool.tile([128, BL * NQROW], F16, tag="attn_sb")

            kvpool = ctx.enter_context(tc.tile_pool(name="kv", bufs=8))
            expool = ctx.enter_context(tc.tile_pool(name="expp", bufs=3))
            attn_ctx = ExitStack()
            lps = attn_ctx.enter_context(tc.tile_pool(name="lps", bufs=4, space="PSUM"))
            atps = attn_ctx.enter_context(tc.tile_pool(name="atps", bufs=1, space="PSUM"))
            dnps = attn_ctx.enter_context(tc.tile_pool(name="dnps", bufs=2, space="PSUM"))

            attn_ps = atps.tile([128, BL * NQROW], F32, tag="attnp")

            kv_tiles = []
            for b in range(BL):
                k16 = kvpool.tile([128, n_tiles * K], F8, tag="k16")
                v16 = kvpool.tile([128, n_tiles * K], F8, tag="v16")
                nc.sync.dma_start(k16[:], ext["kc"].ap()[b])
                nc.sync.dma_start(v16[:], ext["vc"].ap()[b])
                kv_tiles.append((k16, v16))

            for b in range(BL):
                k16, v16 = kv_tiles[b]
                lg = lps.tile([128, LCOLS], F32, tag="lg")
                # kc is host-packed transposed: k16[k, s] -- tile j is cols j*128..
                for j in range(n_tiles):
                    nc.tensor.matmul(lg[:, j * NQROW:(j + 1) * NQROW],
                                     k16[:, j * K:(j + 1) * K],
                                     qT[:, b * NQROW:(b + 1) * NQROW],
                                     skip_group_check=True)

                ex = expool.tile([128, LCOLS], F16, tag="ex")
                nc.scalar.activation(ex[:], lg[:], mybir.ActivationFunctionType.Exp,
                                     bias=c_neg1[:, 0:1], scale=float(SCALE))

                # denominator: ones-matmul partials + strided reduce + fresh part
                dn = dnps.tile([1, LCOLS], F32, tag="dn")
                nc.tensor.matmul(dn[:], ones16[:, 0:1], ex[:], skip_group_check=True)
                nc.vector.reduce_sum(dscr[:],
                                     dn[:].rearrange("p (j q) -> p q j", j=n_tiles),
                                     axis=mybir.AxisListType.X)
                nc.vector.tensor_tensor(denall[:, b * NQROW:(b + 1) * NQROW], dscr[:],
                                        freshden[:, b * NQROW:(b + 1) * NQROW],
                                        mybir.AluOpType.add)

                # A.V accumulation: fresh first, then cache tiles
                nc.tensor.matmul(attn_ps[:, b * NQROW:(b + 1) * NQROW],
                                 vfl_s(b),
                                 expfr[0:T, b * NQROW:(b + 1) * NQROW],
                                 start=True, stop=False, skip_group_check=True)
                for j in range(n_tiles):
                    nc.tensor.matmul(attn_ps[:, b * NQROW:(b + 1) * NQROW],
                                     v16[:, j * K:(j + 1) * K],
                                     ex[:, j * NQROW:(j + 1) * NQROW],
                                     start=False, stop=(j == n_tiles - 1),
                                     skip_group_check=True)

            # ---------------- Normalize + o_proj + output ---------------------
            recip = cpool.tile([1, BL * NQROW], F32, tag="recip")
            rbc = cpool.tile([128, BL * NQROW], F32, tag="rbc")
            nc.vector.reciprocal(recip[:], denall[:])
            bcps = attn_ctx.enter_context(tc.tile_pool(name="bcps", bufs=1, space="PSUM"))
            rbp = bcps.tile([128, BL * NQROW], F32, tag="rbp")
            nc.tensor.matmul(rbp[:], ones_row[:], recip[:])
            nc.scalar.copy(rbc[:], rbp[:])
            # normalize and permute (b r t) -> (r b t) so o_proj lhsT slices are contiguous
            nc.vector.tensor_tensor(
                attn_sb[:].rearrange("p (r b t) -> p b r t", r=R, b=BL),
                attn_ps[:].rearrange("p (b r t) -> p b r t", r=R, b=BL),
                rbc[:].rearrange("p (b r t) -> p b r t", r=R, b=BL),
                mybir.AluOpType.mult)
            attn_ctx.close()

            out_sb = cpool.tile([TOK, D], F32, tag="out_sb")
            with tc.tile_pool(name="ps4", bufs=1, space="PSUM") as ps4, \
                 tc.tile_pool(name="dram", bufs=1, space="DRAM") as dpool:
                outp = ps4.tile([TOK, D], F32, tag="outp")
                if use_collective:
                    bin_ = dpool.tile([TOK, D], F32, tag="bin")
                    bout = dpool.tile([TOK, D], F32, tag="bout")
                # pipelined per-bank: matmuls -> copy -> bounce DMA overlap
                for n in range(D // 512):
                    cs = slice(n * 512, (n + 1) * 512)
                    for r in range(R):
                        nc.tensor.matmul(outp[:, cs],
                                         attn_sb[:, r * TOK:(r + 1) * TOK],
                                         wo[:, r * D + n * 512: r * D + (n + 1) * 512],
                                         start=(r == 0), stop=(r == R - 1))
                    if n % 2 == 0:
                        nc.vector.tensor_copy(out_sb[:, cs], outp[:, cs])
                    else:
                        nc.scalar.copy(out_sb[:, cs], outp[:, cs])
                    if use_collective:
                        nc.sync.dma_start(bin_[:, cs], out_sb[:, cs])
                    else:
                        nc.sync.dma_start(out_ext.ap()[:, cs], out_sb[:, cs])
                if use_collective:
                    nc.gpsimd.collective_compute(
                        "AllReduce",
                        mybir.AluOpType.add,
                        replica_groups=[[0, 1, 2, 3], [4, 5, 6, 7]],
                        ins=[bin_.opt()],
                        outs=[bout.opt()],
                    )
                    nc.sync.dma_start(out_ext.ap(), bout[:])

    nc.compile()
    return nc


def _prepare_inputs(hidden_BTD, segment_ids_BT, k_cache, v_cache, Wq, Wk, Wv, Wo,
                    q_scale, k_scale, cur):
    """Host-side sharding/packing. Returns (in_maps, n_tiles)."""
    hidden = np.asarray(hidden_BTD, np.float32)
    seg = np.asarray(segment_ids_BT)
    kc = np.asarray(k_cache, np.float32)
    vc = np.asarray(v_cache, np.float32)
    Wq = np.asarray(Wq, np.float32)
    Wk = np.asarray(Wk, np.float32)
    Wv = np.asarray(Wv, np.float32)
    Wo = np.asarray(Wo, np.float32)
    q_scale = np.asarray(q_scale, np.float32)
    k_scale = np.asarray(k_scale, np.float32)

    assert cur % 128 == 0 and cur + T <= S, f"unsupported cur_ind {cur}"
    n_tiles = cur // 128

    # positions / pads, exactly as the reference
    valid = (seg != 0)
    csum = np.cumsum(valid.astype(np.int32), axis=-1)
    left_pads = np.sum((csum == 0).astype(np.int32), axis=-1)
    assert np.all(left_pads == 0) and np.all(seg == 1), "only dense segments supported"
    positions = (csum - 1).astype(np.float32) + np.float32(cur)    # (B,T)

    # rope sin/cos in fp32 as the reference computes them
    fraction = np.arange(0, K // 2, dtype=np.float32) * np.float32(2.0 / K)
    timescale = (np.float32(ROPE_BASE) ** fraction).astype(np.float32)
    sinusoid = positions[..., None] / timescale                     # (B,T,64)
    sin = np.sin(sinusoid).astype(np.float32)
    cos = np.cos(sinusoid).astype(np.float32)

    def rope_tables(scale_vec):
        # A[i]: coefficient of x[i]; B[i]: coefficient of x[swap(i)]
        A = np.concatenate([cos * scale_vec[:K // 2], cos * scale_vec[K // 2:]], axis=-1)
        Bc = np.concatenate([-sin * scale_vec[K // 2:], sin * scale_vec[:K // 2]], axis=-1)
        return A.astype(np.float32), Bc.astype(np.float32)          # (B,T,128)

    Aq, Bq = rope_tables(q_scale)
    Ak, Bk = rope_tables(k_scale)

    # fresh-token multiplicative causal mask: (t', b*16 + r*4 + t) -> t' <= t
    m = (np.arange(T)[:, None] <= np.arange(T)[None, :]).astype(np.float16)  # (t',t)
    maskf = np.tile(m[:, None, None, :], (1, BL, R, 1)).reshape(T, BL * R * T)

    in_maps = []
    for c in range(NCORES):
        bh, g = c // 4, c % 4
        bsl = slice(bh * BL, (bh + 1) * BL)
        hT = hidden[bsl].reshape(TOK, D).T                          # (2048, 64)
        qcols = slice(g * QCOLS, (g + 1) * QCOLS)
        kcols = slice(g * K, (g + 1) * K)
        kloc = kc[bsl, :cur, g, :]                                  # (16, cur, 128)
        vloc = vc[bsl, :cur, g, :]
        # K is packed TRANSPOSED (k on partitions): kpack[b, k, s]
        kpack = np.ascontiguousarray(kloc.transpose(0, 2, 1)).astype(F8NP)
        vpack = np.ascontiguousarray(
            vloc.reshape(BL, n_tiles, 128, K).transpose(0, 2, 1, 3).reshape(BL, 128, n_tiles * K)).astype(F8NP)
        aq_l = np.tile(Aq[bsl].reshape(TOK, K), (1, R))
        bq_l = np.tile(Bq[bsl].reshape(TOK, K), (1, R))
        in_maps.append({
            "hT": _pack_rows(np.ascontiguousarray(hT)).astype(np.float16),
            "wq": _pack_rows(np.ascontiguousarray(Wq[:, qcols])).astype(np.float16),
            "wkv": _pack_rows(np.ascontiguousarray(
                np.concatenate([Wk[:, kcols], Wv[:, kcols]], axis=1) * WSCALE)).astype(F8NP),
            "wo": _pack_rows(np.ascontiguousarray(Wo[g * QCOLS:(g + 1) * QCOLS, :])).astype(np.float16),
            "kc": kpack,
            "vc": vpack,
            "aq": np.ascontiguousarray(aq_l),
            "bq": np.ascontiguousarray(bq_l),
            "ak": np.ascontiguousarray(Ak[bsl].reshape(TOK, K)),
            "bk": np.ascontiguousarray(Bk[bsl].reshape(TOK, K)),
            "ident16": np.eye(128, dtype=np.float16),
            "ones16": np.ones((128, 1), np.float16),
            "maskf": maskf,
        })
    return in_maps, n_tiles


def kernel(**inputs):
    cur = int(np.asarray(inputs["cur_ind"]))
    in_maps, n_tiles = _prepare_inputs(
        inputs["hidden_BTD"], inputs["segment_ids_BT"], inputs["k_cache"],
        inputs["v_cache"], inputs["Wq"], inputs["Wk"], inputs["Wv"], inputs["Wo"],
        inputs["q_scale"], inputs["k_scale"], cur)

    for use_coll in (True, False):
        key = (cur, use_coll, 1)
        try:
            if key not in _COMPILED:
                _COMPILED[key] = _build_nc(cur, n_tiles, use_coll, nreps=1)
            nc = _COMPILED[key]
            res = run_bass_kernel_spmd(nc, in_maps, list(range(NCORES)))
            outs = [res.results[c]["out"].reshape(BL, T, D) for c in range(NCORES)]
            if use_coll:
                full = np.concatenate([outs[0], outs[4]], axis=0)
            else:
                full = np.concatenate([sum(outs[0:4]), sum(outs[4:8])], axis=0)
            return full.astype(np.float32)
        except Exception:
            if not use_coll:
                raise
            import traceback
            traceback.print_exc()
            print("collective path failed; falling back to host-side reduce",
                  file=sys.stderr)
    raise RuntimeError("unreachable")



# revision 23
# speedup vs baseline: 867.2357x; 1.1621x over previous
"""Trainium2 Bass kernel for GQA decode attention (B=32,T=4,D=2048,H=16,G=4,K=128,S=4096).

Sharding: 8 NeuronCores = 2 batch-groups x 4 kv-head groups.
Core c: batches [16*(c//4), 16*(c//4)+16), kv head g = c % 4 (owns 4 q heads).
o_proj partial sums are AllReduce'd across each 4-core head group on device.

Device pipeline per core:
  - QKV projection (fp16 matmuls, fp32 PSUM accumulate)
  - RMS-norm + RoPE in fp32 on DVE/ACT (host-precomputed coefficient tables
    with q_scale/k_scale folded in)
  - attention in transposed orientation: PE-transpose K tiles, logits^T =
    kT @ qT (fp16 in, fp32 out), softmax WITHOUT max-subtraction -- safe
    because rms-normed q,k bound |logits| <= sqrt(K); exp computes
    exp(x/sqrt(K) - 1) on ACT (the -1 cancels in normalization and keeps
    fp16 exp values < 65504)
  - A.V accumulates attn^T directly (v natural layout as the stationary)
  - fp16 o_proj into fp32 PSUM, AllReduce, DMA out

Only cache rows [0, cur_ind) are read; rows [cur_ind, cur_ind+T) are the
freshly projected k/v handled on-chip, rows beyond are masked by the
reference -- so the cache update never materializes.
"""

import sys

sys.path.insert(0, "/opt/trn_rl_repo")

import numpy as np
import ml_dtypes

F8NP = ml_dtypes.float8_e3m4
WSCALE = 32.0

import concourse.bacc as bacc
import concourse.mybir as mybir
import concourse.tile as tile
from concourse.bass_utils import run_bass_kernel_spmd

F32 = mybir.dt.float32
F32R = mybir.dt.float32r
F16 = mybir.dt.float16
F8 = mybir.dt.float8e3   # TRN E3M4: ~1.6% normal / 2^-6 denormal step
# WSCALE (defined above): prescale for fp8 qkv weights; rms-norm cancels it
# for q/k, the fresh-v copy divides it back out

B, T, D = 32, 4, 2048
H, G, K = 16, 4, 128
S = 4096
R = H // G          # 4 q heads per kv head
EPS = 1e-6
ROPE_BASE = 10000.0
NCORES = 8
BG = 2              # batch groups
BL = B // BG        # 16 batches per core
TOK = BL * T        # 64 tokens per core
QCOLS = R * K       # 512 local q columns
NDC = D // 128      # 16 contraction chunks for qkv proj
SCALE = 1.0 / np.sqrt(np.float32(K))
EXP_BIAS = -1.0     # exp(x*SCALE + EXP_BIAS); cancels in softmax, avoids fp16 overflow

_COMPILED = {}


def _pack_rows(w):
    """(C*128, N) -> (128, C*N) with [p, c*N+n] = w[c*128+p, n]."""
    c = w.shape[0] // 128
    n = w.shape[1]
    return np.ascontiguousarray(
        w.reshape(c, 128, n).transpose(1, 0, 2).reshape(128, c * n)
    )


def _build_nc(cur, n_tiles, use_collective, nreps=1):
    """Build the kernel program. nreps>1 emits the complete body (all DRAM
    loads + compute + output) back-to-back that many times in one program —
    used by the benchmark harness to measure steady-state per-invocation HW
    time with dispatch overhead amortized. Every rep re-reads all inputs from
    DRAM and rewrites the output, so per-rep work is identical to nreps=1."""
    nc = bacc.Bacc("TRN2", target_bir_lowering=False, debug=False, num_devices=NCORES)

    ext = {}

    def inp(name, shape, dt=F32):
        ext[name] = nc.dram_tensor(name, list(shape), dt, kind="ExternalInput")
        return ext[name]

    inp("hT", (128, NDC * TOK), F16)       # hiddenT packed per d-chunk
    inp("wq", (128, NDC * QCOLS), F16)     # fp16: q noise lands 1:1 on the output
    inp("wkv", (128, NDC * 2 * K), F8)     # [wk|wv] fp8 (affect fresh tokens only)
    inp("wo", (128, R * D), F16)
    inp("kc", (BL, 128, n_tiles * K), F8)  # cache, host-packed (p = s%128), fp8
    inp("vc", (BL, 128, n_tiles * K), F8)  # e3m4: ~1.3% rms; FWL keeps A.V at ~76ns/mm
    inp("aq", (TOK, QCOLS))
    inp("bq", (TOK, QCOLS))
    inp("ak", (TOK, K))
    inp("bk", (TOK, K))
    inp("ident16", (128, 128), F16)
    inp("ones16", (128, 1), F16)
    inp("maskf", (T, BL * 4 * T), F16)     # multiplicative causal mask for fresh tokens
    out_ext = nc.dram_tensor("out", [TOK, D], F16, kind="ExternalOutput")

    NQROW = 4 * T                          # 16 query rows per batch (r*4+t)
    LCOLS = n_tiles * NQROW                # logitsT bank cols per batch

    with tile.TileContext(nc) as tc:
        from contextlib import ExitStack

        for _rep in range(nreps):
          with ExitStack() as ctx:
            cpool = ctx.enter_context(tc.tile_pool(name="const", bufs=1))

            def load(name, dt=None, eng=None, split=1):
                h = ext[name]
                t_ = cpool.tile(list(h.shape), dt or h.dtype, tag=name)
                ncols = h.shape[-1]
                step = ncols // split
                for s0 in range(0, ncols, step):
                    (eng or nc.sync).dma_start(t_[:, s0:s0 + step], h.ap()[:, s0:s0 + step])
                return t_

            hT = load("hT", split=4)
            wq = load("wq", split=4)
            wkv = load("wkv", split=4)
            wo = load("wo", split=2)
            aq = load("aq")
            bq = load("bq")
            ak = load("ak")
            bk = load("bk")
            ident16 = load("ident16")
            ones16 = load("ones16")
            maskf = load("maskf")

            # ---------------- Phase 1: QKV projection + norm + rope ----------
            c_eps = cpool.tile([128, 1], F32, tag="c_eps")
            c_neg1 = cpool.tile([128, 1], F32, tag="c_neg1")
            nc.vector.memset(c_eps[:], float(EPS))
            nc.vector.memset(c_neg1[:], float(EXP_BIAS))
            ones_row = cpool.tile([1, 128], F32, tag="ones_row")
            nc.vector.memset(ones_row[:], 1.0)

            qn = cpool.tile([TOK, QCOLS], F32, tag="qn")       # normed+roped q
            kn = cpool.tile([TOK, K], F32, tag="kn")
            WQKV = QCOLS + 2 * K                               # 768 combined cols
            qkv16 = cpool.tile([TOK, WQKV], F16, tag="qkv16")
            q16 = qkv16[:, 0:QCOLS]
            k16n = qkv16[:, QCOLS:QCOLS + K]
            v16n = qkv16[:, QCOLS + K:WQKV]
            ssq = cpool.tile([TOK, 8], F32, tag="ssq")
            rstd = cpool.tile([TOK, 8], F32, tag="rstd")
            scr = cpool.tile([TOK, QCOLS], F32, tag="scr")
            scr2 = cpool.tile([TOK, QCOLS], F32, tag="scr2")

            with tc.tile_pool(name="ps1", bufs=1, space="PSUM") as ps1:
                pq = ps1.tile([TOK, QCOLS], F32, tag="pq")
                pkv = ps1.tile([TOK, 2 * K], F32, tag="pkv")
                pk = pkv[:, 0:K]
                pv = pkv[:, K:2 * K]
                for c in range(NDC):
                    lhs = hT[:, c * TOK:(c + 1) * TOK]
                    st, sp = (c == 0), (c == NDC - 1)
                    nc.tensor.matmul(pq[:], lhs, wq[:, c * QCOLS:(c + 1) * QCOLS], start=st, stop=sp)
                    nc.tensor.matmul(pkv[:], lhs, wkv[:, c * 2 * K:(c + 1) * 2 * K], start=st, stop=sp)

                # sum of squares per (token, head)
                for h in range(R):
                    nc.scalar.activation(scr[:, h * K:(h + 1) * K], pq[:, h * K:(h + 1) * K],
                                         mybir.ActivationFunctionType.Square,
                                         accum_out=ssq[:, h:h + 1])
                nc.scalar.activation(scr2[:, 0:K], pk,
                                     mybir.ActivationFunctionType.Square,
                                     accum_out=ssq[:, R:R + 1])
                # std = sqrt(ssq/K + eps); rstd = 1/std
                nc.scalar.activation(rstd[:, 0:5], ssq[:, 0:5],
                                     mybir.ActivationFunctionType.Sqrt,
                                     bias=c_eps[0:TOK, 0:1], scale=float(1.0 / K))
                nc.vector.reciprocal(ssq[:, 0:5], rstd[:, 0:5])
                # q_hat = q * rstd (per token/head), same for k
                for h in range(R):
                    nc.vector.tensor_scalar(qn[:, h * K:(h + 1) * K], pq[:, h * K:(h + 1) * K],
                                            ssq[:, h:h + 1], None, mybir.AluOpType.mult)
                nc.vector.tensor_scalar(kn[:], pk, ssq[:, R:R + 1], None, mybir.AluOpType.mult)
                # v -> fp16, undoing the fp8 weight prescale
                nc.scalar.activation(v16n, pv,
                                     mybir.ActivationFunctionType.Copy,
                                     scale=float(1.0 / WSCALE))

            # rope: out = qh*A + swap_halves(qh)*B   (scale folded into A/B)
            def rope(dst16, x, a, b, s1, s2, nh):
                xr = x[:].rearrange("p (h u x) -> p h u x", h=nh, u=2)
                # dst16 is an AP slice of the combined qkv16 tile
                br = b[:].rearrange("p (h u x) -> p h u x", h=nh, u=2)
                s2r = s2[:, 0:nh * K].rearrange("p (h u x) -> p h u x", h=nh, u=2)
                nc.vector.tensor_tensor(s1[:, 0:nh * K], x[:, 0:nh * K], a[:, 0:nh * K], mybir.AluOpType.mult)
                # swapped-half products
                nc.vector.tensor_tensor(s2r[:, :, 0, :], xr[:, :, 1, :], br[:, :, 0, :], mybir.AluOpType.mult)
                nc.vector.tensor_tensor(s2r[:, :, 1, :], xr[:, :, 0, :], br[:, :, 1, :], mybir.AluOpType.mult)
                nc.vector.tensor_tensor(s1[:, 0:nh * K], s1[:, 0:nh * K], s2[:, 0:nh * K], mybir.AluOpType.add)
                nc.vector.tensor_copy(dst16, s1[:, 0:nh * K])

            rope(q16, qn, aq, bq, scr, scr2, R)
            rope(k16n, kn, ak, bk, scr, scr2, 1)

            # flatten only v's (b t) partition layout -> t-partitions (base 0)
            vfl = cpool.tile([T, BL * K], F16, tag="vfl")
            for t in range(T):
                nc.gpsimd.dma_start(vfl[t:t + 1, :], v16n[t::T, :])

            def vfl_s(b):
                return vfl[0:T, b * K:(b + 1) * K]

            # transpose q -> qT (128k x 16 qrow per b), k_new -> kTnew (128k x 4 per b)
            # qT via 4 whole-column transposes (64 tokens at once, base-0 aligned),
            # then one copy permuting (r b t) -> (b r t); kTnew via one transpose.
            qT = cpool.tile([128, BL * NQROW], F16, tag="qT")
            kTnew = cpool.tile([128, BL * T], F16, tag="kTnew")
            with tc.tile_pool(name="ps2", bufs=1, space="PSUM") as ps2:
                qTp = ps2.tile([128, BL * NQROW], F16, tag="qTp")
                kTnp = ps2.tile([128, BL * T], F16, tag="kTnp")
                for r in range(R):
                    nc.tensor.matmul(qTp[:, r * TOK:(r + 1) * TOK],
                                     q16[:, r * K:(r + 1) * K],
                                     ident16[0:TOK, 0:TOK], is_transpose=True,
                                     skip_group_check=True)
                nc.tensor.matmul(kTnp[:], k16n,
                                 ident16[0:TOK, 0:TOK], is_transpose=True,
                                 skip_group_check=True)
                nc.scalar.copy(
                    qT[:].rearrange("p (b r t) -> p b r t", b=BL, r=R),
                    qTp[:].rearrange("p (r b t) -> p b r t", r=R, b=BL))
                nc.scalar.copy(kTnew[:], kTnp[:])

            # ---------------- Fresh-token logits / exp / denom ---------------
            expfr = cpool.tile([T, BL * NQROW], F16, tag="expfr")
            freshden = cpool.tile([1, BL * NQROW], F32, tag="freshden")
            with tc.tile_pool(name="ps3", bufs=1, space="PSUM") as ps3:
                frp = ps3.tile([T, BL * NQROW], F32, tag="frp")
                fdp = ps3.tile([1, BL * NQROW], F32, tag="fdp")
                for b in range(BL):
                    nc.tensor.matmul(frp[0:T, b * NQROW:(b + 1) * NQROW],
                                     kTnew[:, b * T:(b + 1) * T],
                                     qT[:, b * NQROW:(b + 1) * NQROW],
                                     skip_group_check=True)
                nc.scalar.activation(expfr[:], frp[:], mybir.ActivationFunctionType.Exp,
                                     bias=c_neg1[0:T, 0:1], scale=float(SCALE))
                nc.vector.tensor_tensor(expfr[:], expfr[:], maskf[:], mybir.AluOpType.mult)
                nc.tensor.matmul(fdp[:], ones16[0:T, 0:1], expfr[:])
                nc.scalar.copy(freshden[:], fdp[:])

            # ---------------- Main attention loop over batches ----------------
            denall = cpool.tile([1, BL * NQROW], F32, tag="denall")
            dscr = cpool.tile([1, NQROW], F32, tag="dscr")
            attn_sb = cpool.tile([128, BL * NQROW], F16, tag="attn_sb")

            kvpool = ctx.enter_context(tc.tile_pool(name="kv", bufs=8))
            expool = ctx.enter_context(tc.tile_pool(name="expp", bufs=3))
            attn_ctx = ExitStack()
            lps = attn_ctx.enter_context(tc.tile_pool(name="lps", bufs=4, space="PSUM"))
            atps = attn_ctx.enter_context(tc.tile_pool(name="atps", bufs=1, space="PSUM"))
            dnps = attn_ctx.enter_context(tc.tile_pool(name="dnps", bufs=2, space="PSUM"))

            attn_ps = atps.tile([128, BL * NQROW], F32, tag="attnp")

            kv_tiles = []
            for b in range(BL):
                k16 = kvpool.tile([128, n_tiles * K], F8, tag="k16")
                v16 = kvpool.tile([128, n_tiles * K], F8, tag="v16")
                nc.sync.dma_start(k16[:], ext["kc"].ap()[b])
                nc.sync.dma_start(v16[:], ext["vc"].ap()[b])
                kv_tiles.append((k16, v16))

            for b in range(BL):
                k16, v16 = kv_tiles[b]
                lg = lps.tile([128, LCOLS], F32, tag="lg")
                # kc is host-packed transposed: k16[k, s] -- tile j is cols j*128..
                for j in range(n_tiles):
                    nc.tensor.matmul(lg[:, j * NQROW:(j + 1) * NQROW],
                                     k16[:, j * K:(j + 1) * K],
                                     qT[:, b * NQROW:(b + 1) * NQROW],
                                     skip_group_check=True)

                ex = expool.tile([128, LCOLS], F16, tag="ex")
                nc.scalar.activation(ex[:], lg[:], mybir.ActivationFunctionType.Exp,
                                     bias=c_neg1[:, 0:1], scale=float(SCALE))

                # denominator: ones-matmul partials + strided reduce + fresh part
                dn = dnps.tile([1, LCOLS], F32, tag="dn")
                nc.tensor.matmul(dn[:], ones16[:, 0:1], ex[:], skip_group_check=True)
                nc.vector.reduce_sum(dscr[:],
                                     dn[:].rearrange("p (j q) -> p q j", j=n_tiles),
                                     axis=mybir.AxisListType.X)
                nc.vector.tensor_tensor(denall[:, b * NQROW:(b + 1) * NQROW], dscr[:],
                                        freshden[:, b * NQROW:(b + 1) * NQROW],
                                        mybir.AluOpType.add)

                # A.V accumulation: fresh first, then cache tiles
                nc.tensor.matmul(attn_ps[:, b * NQROW:(b + 1) * NQROW],
                                 vfl_s(b),
                                 expfr[0:T, b * NQROW:(b + 1) * NQROW],
                                 start=True, stop=False, skip_group_check=True)
                for j in range(n_tiles):
                    nc.tensor.matmul(attn_ps[:, b * NQROW:(b + 1) * NQROW],
                                     v16[:, j * K:(j + 1) * K],
                                     ex[:, j * NQROW:(j + 1) * NQROW],
                                     start=False, stop=(j == n_tiles - 1),
                                     skip_group_check=True)

            # ---------------- Normalize + o_proj + output ---------------------
            recip = cpool.tile([1, BL * NQROW], F32, tag="recip")
            rbc = cpool.tile([128, BL * NQROW], F32, tag="rbc")
            nc.vector.reciprocal(recip[:], denall[:])
            bcps = attn_ctx.enter_context(tc.tile_pool(name="bcps", bufs=1, space="PSUM"))
            rbp = bcps.tile([128, BL * NQROW], F32, tag="rbp")
            nc.tensor.matmul(rbp[:], ones_row[:], recip[:])
            nc.scalar.copy(rbc[:], rbp[:])
            # normalize and permute (b r t) -> (r b t) so o_proj lhsT slices are contiguous
            nc.vector.tensor_tensor(
                attn_sb[:].rearrange("p (r b t) -> p b r t", r=R, b=BL),
                attn_ps[:].rearrange("p (b r t) -> p b r t", r=R, b=BL),
                rbc[:].rearrange("p (b r t) -> p b r t", r=R, b=BL),
                mybir.AluOpType.mult)
            attn_ctx.close()

            out_sb = cpool.tile([TOK, D], F16, tag="out_sb")
            with tc.tile_pool(name="ps4", bufs=1, space="PSUM") as ps4, \
                 tc.tile_pool(name="dram", bufs=1, space="DRAM") as dpool:
                outp = ps4.tile([TOK, D], F32, tag="outp")
                if use_collective:
                    bin_ = dpool.tile([TOK, D], F16, tag="bin")
                    bout = dpool.tile([TOK, D], F16, tag="bout")
                # pipelined per-bank: matmuls -> copy -> bounce DMA overlap
                for n in range(D // 512):
                    cs = slice(n * 512, (n + 1) * 512)
                    for r in range(R):
                        nc.tensor.matmul(outp[:, cs],
                                         attn_sb[:, r * TOK:(r + 1) * TOK],
                                         wo[:, r * D + n * 512: r * D + (n + 1) * 512],
                                         start=(r == 0), stop=(r == R - 1))
                    if n % 2 == 0:
                        nc.vector.tensor_copy(out_sb[:, cs], outp[:, cs])
                    else:
                        nc.scalar.copy(out_sb[:, cs], outp[:, cs])
                    if use_collective:
                        nc.sync.dma_start(bin_[:, cs], out_sb[:, cs])
                    else:
                        nc.sync.dma_start(out_ext.ap()[:, cs], out_sb[:, cs])
                if use_collective:
                    nc.gpsimd.collective_compute(
                        "AllReduce",
                        mybir.AluOpType.add,
                        replica_groups=[[0, 1, 2, 3], [4, 5, 6, 7]],
                        ins=[bin_.opt()],
                        outs=[bout.opt()],
                    )
                    nc.sync.dma_start(out_ext.ap(), bout[:])

    nc.compile()
    return nc


def _prepare_inputs(hidden_BTD, segment_ids_BT, k_cache, v_cache, Wq, Wk, Wv, Wo,
                    q_scale, k_scale, cur):
    """Host-side sharding/packing. Returns (in_maps, n_tiles)."""
    hidden = np.asarray(hidden_BTD, np.float32)
    seg = np.asarray(segment_ids_BT)
    kc = np.asarray(k_cache, np.float32)
    vc = np.asarray(v_cache, np.float32)
    Wq = np.asarray(Wq, np.float32)
    Wk = np.asarray(Wk, np.float32)
    Wv = np.asarray(Wv, np.float32)
    Wo = np.asarray(Wo, np.float32)
    q_scale = np.asarray(q_scale, np.float32)
    k_scale = np.asarray(k_scale, np.float32)

    assert cur % 128 == 0 and cur + T <= S, f"unsupported cur_ind {cur}"
    n_tiles = cur // 128

    # positions / pads, exactly as the reference
    valid = (seg != 0)
    csum = np.cumsum(valid.astype(np.int32), axis=-1)
    left_pads = np.sum((csum == 0).astype(np.int32), axis=-1)
    assert np.all(left_pads == 0) and np.all(seg == 1), "only dense segments supported"
    positions = (csum - 1).astype(np.float32) + np.float32(cur)    # (B,T)

    # rope sin/cos in fp32 as the reference computes them
    fraction = np.arange(0, K // 2, dtype=np.float32) * np.float32(2.0 / K)
    timescale = (np.float32(ROPE_BASE) ** fraction).astype(np.float32)
    sinusoid = positions[..., None] / timescale                     # (B,T,64)
    sin = np.sin(sinusoid).astype(np.float32)
    cos = np.cos(sinusoid).astype(np.float32)

    def rope_tables(scale_vec):
        # A[i]: coefficient of x[i]; B[i]: coefficient of x[swap(i)]
        A = np.concatenate([cos * scale_vec[:K // 2], cos * scale_vec[K // 2:]], axis=-1)
        Bc = np.concatenate([-sin * scale_vec[K // 2:], sin * scale_vec[:K // 2]], axis=-1)
        return A.astype(np.float32), Bc.astype(np.float32)          # (B,T,128)

    Aq, Bq = rope_tables(q_scale)
    Ak, Bk = rope_tables(k_scale)

    # fresh-token multiplicative causal mask: (t', b*16 + r*4 + t) -> t' <= t
    m = (np.arange(T)[:, None] <= np.arange(T)[None, :]).astype(np.float16)  # (t',t)
    maskf = np.tile(m[:, None, None, :], (1, BL, R, 1)).reshape(T, BL * R * T)

    in_maps = []
    for c in range(NCORES):
        bh, g = c // 4, c % 4
        bsl = slice(bh * BL, (bh + 1) * BL)
        hT = hidden[bsl].reshape(TOK, D).T                          # (2048, 64)
        qcols = slice(g * QCOLS, (g + 1) * QCOLS)
        kcols = slice(g * K, (g + 1) * K)
        kloc = kc[bsl, :cur, g, :]                                  # (16, cur, 128)
        vloc = vc[bsl, :cur, g, :]
        # K is packed TRANSPOSED (k on partitions): kpack[b, k, s]
        kpack = np.ascontiguousarray(kloc.transpose(0, 2, 1)).astype(F8NP)
        vpack = np.ascontiguousarray(
            vloc.reshape(BL, n_tiles, 128, K).transpose(0, 2, 1, 3).reshape(BL, 128, n_tiles * K)).astype(F8NP)
        aq_l = np.tile(Aq[bsl].reshape(TOK, K), (1, R))
        bq_l = np.tile(Bq[bsl].reshape(TOK, K), (1, R))
        in_maps.append({
            "hT": _pack_rows(np.ascontiguousarray(hT)).astype(np.float16),
            "wq": _pack_rows(np.ascontiguousarray(Wq[:, qcols])).astype(np.float16),
            "wkv": _pack_rows(np.ascontiguousarray(
                np.concatenate([Wk[:, kcols], Wv[:, kcols]], axis=1) * WSCALE)).astype(F8NP),
            "wo": _pack_rows(np.ascontiguousarray(Wo[g * QCOLS:(g + 1) * QCOLS, :])).astype(np.float16),
            "kc": kpack,
            "vc": vpack,
            "aq": np.ascontiguousarray(aq_l),
            "bq": np.ascontiguousarray(bq_l),
            "ak": np.ascontiguousarray(Ak[bsl].reshape(TOK, K)),
            "bk": np.ascontiguousarray(Bk[bsl].reshape(TOK, K)),
            "ident16": np.eye(128, dtype=np.float16),
            "ones16": np.ones((128, 1), np.float16),
            "maskf": maskf,
        })
    return in_maps, n_tiles


def kernel(**inputs):
    cur = int(np.asarray(inputs["cur_ind"]))
    in_maps, n_tiles = _prepare_inputs(
        inputs["hidden_BTD"], inputs["segment_ids_BT"], inputs["k_cache"],
        inputs["v_cache"], inputs["Wq"], inputs["Wk"], inputs["Wv"], inputs["Wo"],
        inputs["q_scale"], inputs["k_scale"], cur)

    for use_coll in (True, False):
        key = (cur, use_coll, 1)
        try:
            if key not in _COMPILED:
                _COMPILED[key] = _build_nc(cur, n_tiles, use_coll, nreps=1)
            nc = _COMPILED[key]
            res = run_bass_kernel_spmd(nc, in_maps, list(range(NCORES)))
            outs = [res.results[c]["out"].astype(np.float32).reshape(BL, T, D)
                    for c in range(NCORES)]
            if use_coll:
                full = np.concatenate([outs[0], outs[4]], axis=0)
            else:
                full = np.concatenate([sum(outs[0:4]), sum(outs[4:8])], axis=0)
            return full.astype(np.float32)
        except Exception:
            if not use_coll:
                raise
            import traceback
            traceback.print_exc()
            print("collective path failed; falling back to host-side reduce",
                  file=sys.stderr)
    raise RuntimeError("unreachable")



# revision 27
# speedup vs baseline: 899.6435x; 1.0374x over previous
"""Trainium2 Bass kernel for GQA decode attention (B=32,T=4,D=2048,H=16,G=4,K=128,S=4096).

Sharding: 8 NeuronCores = 2 batch-groups x 4 kv-head groups.
Core c: batches [16*(c//4), 16*(c//4)+16), kv head g = c % 4 (owns 4 q heads).
o_proj partial sums are AllReduce'd across each 4-core head group on device.

Device pipeline per core:
  - QKV projection (fp16 matmuls, merged Wk|Wv tile, fp32 PSUM)
  - RMS-norm + RoPE in fp32 on DVE/ACT (host-precomputed coefficient tables
    with q_scale/k_scale folded in)
  - attention in transposed orientation: logits^T = kT(fp8) @ qT(fp16),
    softmax WITHOUT max-subtraction -- safe because rms-normed q,k bound
    |logits| <= sqrt(K); exp computes exp(x/sqrt(K) - 1) on ACT (the -1
    cancels in normalization and keeps fp16 exp values < 65504)
  - A.V accumulates attn^T directly (fp8 v stationary -- 128-col fp8
    stationaries engage FWL, ~76ns/matmul vs ~130ns fp16)
  - fp16 o_proj into fp32 PSUM, fp16 AllReduce bounce, fp16 DMA out

Precision: the k/v cache is TRN E3M4 fp8 (~1.3% rms quantization on N(0,1)
data). Logit/value noise lands ~1:1 on the output here (the attention
output is itself a ~1/sqrt(Neff)-scale weighted average), so e2e rel err is
~1.7e-2 vs the 2e-2 gate. All weights stay fp16: their noise also lands 1:1
and fp8 there would blow the budget (and fp8 as a PE moving operand is slow
anyway).

Only cache rows [0, cur_ind) are read; rows [cur_ind, cur_ind+T) are the
freshly projected k/v handled on-chip, rows beyond are masked by the
reference -- so the cache update never materializes.
"""

import sys

sys.path.insert(0, "/opt/trn_rl_repo")

import numpy as np
import ml_dtypes

F8NP = ml_dtypes.float8_e3m4

import concourse.bacc as bacc
import concourse.mybir as mybir
import concourse.tile as tile
from concourse.bass_utils import run_bass_kernel_spmd

F32 = mybir.dt.float32
F32R = mybir.dt.float32r
F16 = mybir.dt.float16
F8 = mybir.dt.float8e3   # TRN E3M4: ~1.6% normal-range / 2^-6 denormal step

B, T, D = 32, 4, 2048
H, G, K = 16, 4, 128
S = 4096
R = H // G          # 4 q heads per kv head
EPS = 1e-6
ROPE_BASE = 10000.0
NCORES = 8
BG = 2              # batch groups
BL = B // BG        # 16 batches per core
TOK = BL * T        # 64 tokens per core
QCOLS = R * K       # 512 local q columns
NDC = D // 128      # 16 contraction chunks for qkv proj
SCALE = 1.0 / np.sqrt(np.float32(K))
EXP_BIAS = -1.0     # exp(x*SCALE + EXP_BIAS); cancels in softmax, avoids fp16 overflow

_COMPILED = {}


def _pack_rows(w):
    """(C*128, N) -> (128, C*N) with [p, c*N+n] = w[c*128+p, n]."""
    c = w.shape[0] // 128
    n = w.shape[1]
    return np.ascontiguousarray(
        w.reshape(c, 128, n).transpose(1, 0, 2).reshape(128, c * n)
    )


def _build_nc(cur, n_tiles, use_collective, nreps=1):
    """Build the kernel program. nreps>1 emits the complete body (all DRAM
    loads + compute + output) back-to-back that many times in one program —
    used by the benchmark harness to measure steady-state per-invocation HW
    time with dispatch overhead amortized. Every rep re-reads all inputs from
    DRAM and rewrites the output, so per-rep work is identical to nreps=1."""
    nc = bacc.Bacc("TRN2", target_bir_lowering=False, debug=False, num_devices=NCORES)

    ext = {}

    def inp(name, shape, dt=F32):
        ext[name] = nc.dram_tensor(name, list(shape), dt, kind="ExternalInput")
        return ext[name]

    inp("hT", (128, NDC * TOK), F16)       # hiddenT packed per d-chunk
    inp("wq", (128, NDC * QCOLS), F16)     # fp16: q noise lands 1:1 on the output
    inp("wkv", (128, NDC * 2 * K), F16)    # [wk|wv] merged; fp16 moving (fp8 moving is slow on PE)
    inp("wo", (128, R * D), F16)
    inp("kc", (BL, 128, n_tiles * K), F8)  # cache, host-packed (p = s%128), fp8
    inp("vc", (BL, 128, n_tiles * K), F8)  # e3m4: ~1.3% rms; FWL keeps A.V at ~76ns/mm
    inp("aq", (TOK, QCOLS))
    inp("bq", (TOK, QCOLS))
    inp("ak", (TOK, K))
    inp("bk", (TOK, K))
    inp("ident16", (128, 128), F16)
    inp("ones16", (128, 1), F16)
    inp("maskf", (T, BL * 4 * T), F16)     # multiplicative causal mask for fresh tokens
    out_ext = nc.dram_tensor("out", [TOK, D], F16, kind="ExternalOutput")

    NQROW = 4 * T                          # 16 query rows per batch (r*4+t)
    LCOLS = n_tiles * NQROW                # logitsT bank cols per batch

    with tile.TileContext(nc) as tc:
        from contextlib import ExitStack

        for _rep in range(nreps):
          with ExitStack() as ctx:
            cpool = ctx.enter_context(tc.tile_pool(name="const", bufs=1))

            def load(name, dt=None, eng=None, split=1):
                h = ext[name]
                t_ = cpool.tile(list(h.shape), dt or h.dtype, tag=name)
                ncols = h.shape[-1]
                step = ncols // split
                for s0 in range(0, ncols, step):
                    (eng or nc.sync).dma_start(t_[:, s0:s0 + step], h.ap()[:, s0:s0 + step])
                return t_

            hT = load("hT", split=4)
            wq = load("wq", split=4)
            wkv = load("wkv", split=4)
            wo = load("wo", split=2)
            aq = load("aq")
            bq = load("bq")
            ak = load("ak")
            bk = load("bk")
            ident16 = load("ident16")
            ones16 = load("ones16")
            maskf = load("maskf")

            # ---------------- Phase 1: QKV projection + norm + rope ----------
            c_eps = cpool.tile([128, 1], F32, tag="c_eps")
            c_neg1 = cpool.tile([128, 1], F32, tag="c_neg1")
            nc.vector.memset(c_eps[:], float(EPS))
            nc.vector.memset(c_neg1[:], float(EXP_BIAS))
            ones_row = cpool.tile([1, 128], F32, tag="ones_row")
            nc.vector.memset(ones_row[:], 1.0)

            qn = cpool.tile([TOK, QCOLS], F32, tag="qn")       # normed+roped q
            kn = cpool.tile([TOK, K], F32, tag="kn")
            WQKV = QCOLS + 2 * K                               # 768 combined cols
            qkv16 = cpool.tile([TOK, WQKV], F16, tag="qkv16")
            q16 = qkv16[:, 0:QCOLS]
            k16n = qkv16[:, QCOLS:QCOLS + K]
            v16n = qkv16[:, QCOLS + K:WQKV]
            ssq = cpool.tile([TOK, 8], F32, tag="ssq")
            rstd = cpool.tile([TOK, 8], F32, tag="rstd")
            scr = cpool.tile([TOK, QCOLS], F32, tag="scr")
            scr2 = cpool.tile([TOK, QCOLS], F32, tag="scr2")

            with tc.tile_pool(name="ps1", bufs=1, space="PSUM") as ps1:
                pq = ps1.tile([TOK, QCOLS], F32, tag="pq")
                pkv = ps1.tile([TOK, 2 * K], F32, tag="pkv")
                pk = pkv[:, 0:K]
                pv = pkv[:, K:2 * K]
                for c in range(NDC):
                    lhs = hT[:, c * TOK:(c + 1) * TOK]
                    st, sp = (c == 0), (c == NDC - 1)
                    nc.tensor.matmul(pq[:], lhs, wq[:, c * QCOLS:(c + 1) * QCOLS], start=st, stop=sp)
                    nc.tensor.matmul(pkv[:], lhs, wkv[:, c * 2 * K:(c + 1) * 2 * K], start=st, stop=sp)

                # sum of squares per (token, head)
                for h in range(R):
                    nc.scalar.activation(scr[:, h * K:(h + 1) * K], pq[:, h * K:(h + 1) * K],
                                         mybir.ActivationFunctionType.Square,
                                         accum_out=ssq[:, h:h + 1])
                nc.scalar.activation(scr2[:, 0:K], pk,
                                     mybir.ActivationFunctionType.Square,
                                     accum_out=ssq[:, R:R + 1])
                # std = sqrt(ssq/K + eps); rstd = 1/std
                nc.scalar.activation(rstd[:, 0:5], ssq[:, 0:5],
                                     mybir.ActivationFunctionType.Sqrt,
                                     bias=c_eps[0:TOK, 0:1], scale=float(1.0 / K))
                nc.vector.reciprocal(ssq[:, 0:5], rstd[:, 0:5])
                # q_hat = q * rstd (per token/head), same for k
                for h in range(R):
                    nc.vector.tensor_scalar(qn[:, h * K:(h + 1) * K], pq[:, h * K:(h + 1) * K],
                                            ssq[:, h:h + 1], None, mybir.AluOpType.mult)
                nc.vector.tensor_scalar(kn[:], pk, ssq[:, R:R + 1], None, mybir.AluOpType.mult)
                # v -> fp16
                nc.scalar.copy(v16n, pv)

            # rope: out = qh*A + swap_halves(qh)*B   (scale folded into A/B)
            def rope(dst16, x, a, b, s1, s2, nh):
                xr = x[:].rearrange("p (h u x) -> p h u x", h=nh, u=2)
                # dst16 is an AP slice of the combined qkv16 tile
                br = b[:].rearrange("p (h u x) -> p h u x", h=nh, u=2)
                s2r = s2[:, 0:nh * K].rearrange("p (h u x) -> p h u x", h=nh, u=2)
                nc.vector.tensor_tensor(s1[:, 0:nh * K], x[:, 0:nh * K], a[:, 0:nh * K], mybir.AluOpType.mult)
                # swapped-half products
                nc.vector.tensor_tensor(s2r[:, :, 0, :], xr[:, :, 1, :], br[:, :, 0, :], mybir.AluOpType.mult)
                nc.vector.tensor_tensor(s2r[:, :, 1, :], xr[:, :, 0, :], br[:, :, 1, :], mybir.AluOpType.mult)
                nc.vector.tensor_tensor(s1[:, 0:nh * K], s1[:, 0:nh * K], s2[:, 0:nh * K], mybir.AluOpType.add)
                nc.vector.tensor_copy(dst16, s1[:, 0:nh * K])

            rope(q16, qn, aq, bq, scr, scr2, R)
            rope(k16n, kn, ak, bk, scr, scr2, 1)

            # flatten only v's (b t) partition layout -> t-partitions (base 0)
            vfl = cpool.tile([T, BL * K], F16, tag="vfl")
            for t in range(T):
                nc.gpsimd.dma_start(vfl[t:t + 1, :], v16n[t::T, :])

            def vfl_s(b):
                return vfl[0:T, b * K:(b + 1) * K]

            # transpose q -> qT (128k x 16 qrow per b), k_new -> kTnew (128k x 4 per b)
            # qT via 4 whole-column transposes (64 tokens at once, base-0 aligned),
            # then one copy permuting (r b t) -> (b r t); kTnew via one transpose.
            qT = cpool.tile([128, BL * NQROW], F16, tag="qT")
            kTnew = cpool.tile([128, BL * T], F16, tag="kTnew")
            with tc.tile_pool(name="ps2", bufs=1, space="PSUM") as ps2:
                qTp = ps2.tile([128, BL * NQROW], F16, tag="qTp")
                kTnp = ps2.tile([128, BL * T], F16, tag="kTnp")
                for r in range(R):
                    nc.tensor.matmul(qTp[:, r * TOK:(r + 1) * TOK],
                                     q16[:, r * K:(r + 1) * K],
                                     ident16[0:TOK, 0:TOK], is_transpose=True,
                                     skip_group_check=True)
                nc.tensor.matmul(kTnp[:], k16n,
                                 ident16[0:TOK, 0:TOK], is_transpose=True,
                                 skip_group_check=True)
                nc.scalar.copy(
                    qT[:].rearrange("p (b r t) -> p b r t", b=BL, r=R),
                    qTp[:].rearrange("p (r b t) -> p b r t", r=R, b=BL))
                nc.scalar.copy(kTnew[:], kTnp[:])

            # ---------------- Fresh-token logits / exp / denom ---------------
            expfr = cpool.tile([T, BL * NQROW], F16, tag="expfr")
            freshden = cpool.tile([1, BL * NQROW], F32, tag="freshden")
            with tc.tile_pool(name="ps3", bufs=1, space="PSUM") as ps3:
                frp = ps3.tile([T, BL * NQROW], F32, tag="frp")
                fdp = ps3.tile([1, BL * NQROW], F32, tag="fdp")
                for b in range(BL):
                    nc.tensor.matmul(frp[0:T, b * NQROW:(b + 1) * NQROW],
                                     kTnew[:, b * T:(b + 1) * T],
                                     qT[:, b * NQROW:(b + 1) * NQROW],
                                     skip_group_check=True)
                nc.scalar.activation(expfr[:], frp[:], mybir.ActivationFunctionType.Exp,
                                     bias=c_neg1[0:T, 0:1], scale=float(SCALE))
                nc.vector.tensor_tensor(expfr[:], expfr[:], maskf[:], mybir.AluOpType.mult)
                nc.tensor.matmul(fdp[:], ones16[0:T, 0:1], expfr[:])
                nc.scalar.copy(freshden[:], fdp[:])

            # ---------------- Main attention loop over batches ----------------
            denall = cpool.tile([1, BL * NQROW], F32, tag="denall")
            dscr = cpool.tile([1, NQROW], F32, tag="dscr")
            attn_sb = cpool.tile([128, BL * NQROW], F16, tag="attn_sb")

            kvpool = ctx.enter_context(tc.tile_pool(name="kv", bufs=8))
            expool = ctx.enter_context(tc.tile_pool(name="expp", bufs=3))
            attn_ctx = ExitStack()
            lps = attn_ctx.enter_context(tc.tile_pool(name="lps", bufs=4, space="PSUM"))
            atps = attn_ctx.enter_context(tc.tile_pool(name="atps", bufs=1, space="PSUM"))
            dnps = attn_ctx.enter_context(tc.tile_pool(name="dnps", bufs=2, space="PSUM"))

            attn_ps = atps.tile([128, BL * NQROW], F32, tag="attnp")

            kv_tiles = []
            for b in range(BL):
                k16 = kvpool.tile([128, n_tiles * K], F8, tag="k16")
                v16 = kvpool.tile([128, n_tiles * K], F8, tag="v16")
                nc.sync.dma_start(k16[:], ext["kc"].ap()[b])
                nc.sync.dma_start(v16[:], ext["vc"].ap()[b])
                kv_tiles.append((k16, v16))

            for b in range(BL):
                k16, v16 = kv_tiles[b]
                lg = lps.tile([128, LCOLS], F32, tag="lg")
                # kc is host-packed transposed: k16[k, s] -- tile j is cols j*128..
                for j in range(n_tiles):
                    nc.tensor.matmul(lg[:, j * NQROW:(j + 1) * NQROW],
                                     k16[:, j * K:(j + 1) * K],
                                     qT[:, b * NQROW:(b + 1) * NQROW],
                                     skip_group_check=True)

                ex = expool.tile([128, LCOLS], F16, tag="ex")
                nc.scalar.activation(ex[:], lg[:], mybir.ActivationFunctionType.Exp,
                                     bias=c_neg1[:, 0:1], scale=float(SCALE))

                # denominator: ones-matmul partials + strided reduce + fresh part
                dn = dnps.tile([1, LCOLS], F32, tag="dn")
                nc.tensor.matmul(dn[:], ones16[:, 0:1], ex[:], skip_group_check=True)
                nc.vector.reduce_sum(dscr[:],
                                     dn[:].rearrange("p (j q) -> p q j", j=n_tiles),
                                     axis=mybir.AxisListType.X)
                nc.vector.tensor_tensor(denall[:, b * NQROW:(b + 1) * NQROW], dscr[:],
                                        freshden[:, b * NQROW:(b + 1) * NQROW],
                                        mybir.AluOpType.add)

                # A.V accumulation: cache tiles first; the fresh matmul goes
                # LAST so the slow strided vfl gather is off the critical path
                for j in range(n_tiles):
                    nc.tensor.matmul(attn_ps[:, b * NQROW:(b + 1) * NQROW],
                                     v16[:, j * K:(j + 1) * K],
                                     ex[:, j * NQROW:(j + 1) * NQROW],
                                     start=(j == 0), stop=False,
                                     skip_group_check=True)
                nc.tensor.matmul(attn_ps[:, b * NQROW:(b + 1) * NQROW],
                                 vfl_s(b),
                                 expfr[0:T, b * NQROW:(b + 1) * NQROW],
                                 start=False, stop=True, skip_group_check=True)

            # ---------------- Normalize + o_proj + output ---------------------
            recip = cpool.tile([1, BL * NQROW], F32, tag="recip")
            rbc = cpool.tile([128, BL * NQROW], F32, tag="rbc")
            nc.vector.reciprocal(recip[:], denall[:])
            bcps = attn_ctx.enter_context(tc.tile_pool(name="bcps", bufs=1, space="PSUM"))
            rbp = bcps.tile([128, BL * NQROW], F32, tag="rbp")
            nc.tensor.matmul(rbp[:], ones_row[:], recip[:])
            nc.scalar.copy(rbc[:], rbp[:])
            # normalize and permute (b r t) -> (r b t) so o_proj lhsT slices are contiguous
            nc.vector.tensor_tensor(
                attn_sb[:].rearrange("p (r b t) -> p b r t", r=R, b=BL),
                attn_ps[:].rearrange("p (b r t) -> p b r t", r=R, b=BL),
                rbc[:].rearrange("p (b r t) -> p b r t", r=R, b=BL),
                mybir.AluOpType.mult)
            attn_ctx.close()

            out_sb = cpool.tile([TOK, D], F16, tag="out_sb")
            with tc.tile_pool(name="ps4", bufs=1, space="PSUM") as ps4, \
                 tc.tile_pool(name="dram", bufs=1, space="DRAM") as dpool:
                outp = ps4.tile([TOK, D], F32, tag="outp")
                if use_collective:
                    bin_ = dpool.tile([TOK, D], F16, tag="bin")
                    bout = dpool.tile([TOK, D], F16, tag="bout")
                # pipelined per-bank: matmuls -> copy -> bounce DMA overlap
                for n in range(D // 512):
                    cs = slice(n * 512, (n + 1) * 512)
                    for r in range(R):
                        nc.tensor.matmul(outp[:, cs],
                                         attn_sb[:, r * TOK:(r + 1) * TOK],
                                         wo[:, r * D + n * 512: r * D + (n + 1) * 512],
                                         start=(r == 0), stop=(r == R - 1))
                    if n % 2 == 0:
                        nc.vector.tensor_copy(out_sb[:, cs], outp[:, cs])
                    else:
                        nc.scalar.copy(out_sb[:, cs], outp[:, cs])
                    if use_collective:
                        nc.sync.dma_start(bin_[:, cs], out_sb[:, cs])
                    else:
                        nc.sync.dma_start(out_ext.ap()[:, cs], out_sb[:, cs])
                if use_collective:
                    nc.gpsimd.collective_compute(
                        "AllReduce",
                        mybir.AluOpType.add,
                        replica_groups=[[0, 1, 2, 3], [4, 5, 6, 7]],
                        ins=[bin_.opt()],
                        outs=[bout.opt()],
                    )
                    nc.sync.dma_start(out_ext.ap(), bout[:])

    nc.compile()
    return nc


def _prepare_inputs(hidden_BTD, segment_ids_BT, k_cache, v_cache, Wq, Wk, Wv, Wo,
                    q_scale, k_scale, cur):
    """Host-side sharding/packing. Returns (in_maps, n_tiles)."""
    hidden = np.asarray(hidden_BTD, np.float32)
    seg = np.asarray(segment_ids_BT)
    kc = np.asarray(k_cache, np.float32)
    vc = np.asarray(v_cache, np.float32)
    Wq = np.asarray(Wq, np.float32)
    Wk = np.asarray(Wk, np.float32)
    Wv = np.asarray(Wv, np.float32)
    Wo = np.asarray(Wo, np.float32)
    q_scale = np.asarray(q_scale, np.float32)
    k_scale = np.asarray(k_scale, np.float32)

    assert cur % 128 == 0 and cur + T <= S, f"unsupported cur_ind {cur}"
    n_tiles = cur // 128

    # positions / pads, exactly as the reference
    valid = (seg != 0)
    csum = np.cumsum(valid.astype(np.int32), axis=-1)
    left_pads = np.sum((csum == 0).astype(np.int32), axis=-1)
    assert np.all(left_pads == 0) and np.all(seg == 1), "only dense segments supported"
    positions = (csum - 1).astype(np.float32) + np.float32(cur)    # (B,T)

    # rope sin/cos in fp32 as the reference computes them
    fraction = np.arange(0, K // 2, dtype=np.float32) * np.float32(2.0 / K)
    timescale = (np.float32(ROPE_BASE) ** fraction).astype(np.float32)
    sinusoid = positions[..., None] / timescale                     # (B,T,64)
    sin = np.sin(sinusoid).astype(np.float32)
    cos = np.cos(sinusoid).astype(np.float32)

    def rope_tables(scale_vec):
        # A[i]: coefficient of x[i]; B[i]: coefficient of x[swap(i)]
        A = np.concatenate([cos * scale_vec[:K // 2], cos * scale_vec[K // 2:]], axis=-1)
        Bc = np.concatenate([-sin * scale_vec[K // 2:], sin * scale_vec[:K // 2]], axis=-1)
        return A.astype(np.float32), Bc.astype(np.float32)          # (B,T,128)

    Aq, Bq = rope_tables(q_scale)
    Ak, Bk = rope_tables(k_scale)

    # fresh-token multiplicative causal mask: (t', b*16 + r*4 + t) -> t' <= t
    m = (np.arange(T)[:, None] <= np.arange(T)[None, :]).astype(np.float16)  # (t',t)
    maskf = np.tile(m[:, None, None, :], (1, BL, R, 1)).reshape(T, BL * R * T)

    in_maps = []
    for c in range(NCORES):
        bh, g = c // 4, c % 4
        bsl = slice(bh * BL, (bh + 1) * BL)
        hT = hidden[bsl].reshape(TOK, D).T                          # (2048, 64)
        qcols = slice(g * QCOLS, (g + 1) * QCOLS)
        kcols = slice(g * K, (g + 1) * K)
        kloc = kc[bsl, :cur, g, :]                                  # (16, cur, 128)
        vloc = vc[bsl, :cur, g, :]
        # K is packed TRANSPOSED (k on partitions): kpack[b, k, s]
        kpack = np.ascontiguousarray(kloc.transpose(0, 2, 1)).astype(F8NP)
        vpack = np.ascontiguousarray(
            vloc.reshape(BL, n_tiles, 128, K).transpose(0, 2, 1, 3).reshape(BL, 128, n_tiles * K)).astype(F8NP)
        aq_l = np.tile(Aq[bsl].reshape(TOK, K), (1, R))
        bq_l = np.tile(Bq[bsl].reshape(TOK, K), (1, R))
        in_maps.append({
            "hT": _pack_rows(np.ascontiguousarray(hT)).astype(np.float16),
            "wq": _pack_rows(np.ascontiguousarray(Wq[:, qcols])).astype(np.float16),
            "wkv": _pack_rows(np.ascontiguousarray(
                np.concatenate([Wk[:, kcols], Wv[:, kcols]], axis=1))).astype(np.float16),
            "wo": _pack_rows(np.ascontiguousarray(Wo[g * QCOLS:(g + 1) * QCOLS, :])).astype(np.float16),
            "kc": kpack,
            "vc": vpack,
            "aq": np.ascontiguousarray(aq_l),
            "bq": np.ascontiguousarray(bq_l),
            "ak": np.ascontiguousarray(Ak[bsl].reshape(TOK, K)),
            "bk": np.ascontiguousarray(Bk[bsl].reshape(TOK, K)),
            "ident16": np.eye(128, dtype=np.float16),
            "ones16": np.ones((128, 1), np.float16),
            "maskf": maskf,
        })
    return in_maps, n_tiles


def kernel(**inputs):
    cur = int(np.asarray(inputs["cur_ind"]))
    in_maps, n_tiles = _prepare_inputs(
        inputs["hidden_BTD"], inputs["segment_ids_BT"], inputs["k_cache"],
        inputs["v_cache"], inputs["Wq"], inputs["Wk"], inputs["Wv"], inputs["Wo"],
        inputs["q_scale"], inputs["k_scale"], cur)

    for use_coll in (True, False):
        key = (cur, use_coll, 1)
        try:
            if key not in _COMPILED:
                _COMPILED[key] = _build_nc(cur, n_tiles, use_coll, nreps=1)
            nc = _COMPILED[key]
            res = run_bass_kernel_spmd(nc, in_maps, list(range(NCORES)))
            outs = [res.results[c]["out"].astype(np.float32).reshape(BL, T, D)
                    for c in range(NCORES)]
            if use_coll:
                full = np.concatenate([outs[0], outs[4]], axis=0)
            else:
                full = np.concatenate([sum(outs[0:4]), sum(outs[4:8])], axis=0)
            return full.astype(np.float32)
        except Exception:
            if not use_coll:
                raise
            import traceback
            traceback.print_exc()
            print("collective path failed; falling back to host-side reduce",
                  file=sys.stderr)
    raise RuntimeError("unreachable")

